# revision 1
# baseline (speedup 1.0000x reference)
"""Self-contained 2-layer GAT kernel for Trainium2 (8 NeuronCores).

Destination-sharded: each core owns 12544 destination nodes. Edges (+self
loops) are bucketed by (dst-block of 128, src-range of 25088) on the host and
padded to one shared static schedule so a single SPMD program serves all 8
cores. Per 128-edge block a one-hot selection matrix (is_equal vs iota) turns
the segment softmax + scatter-add into PE matmuls accumulated in PSUM; the
softmax is computed unnormalized (exp, exact normalization by the accumulated
denominator). Per-source feature rows are fetched with the int16 SWDGE
dma_gather (mlp ucode library) from 4 sub-tables of 25088 rows. Layer outputs
are destination shards; the host concatenates them between the two launches.
"""
import sys
sys.path.insert(0, '/opt/trn_rl_repo')
import time
import numpy as np
import jax
from jax.sharding import Mesh, PartitionSpec
from jax.experimental.shard_map import shard_map

import concourse.bass as bass
import concourse.tile as tile
from concourse import bacc, mybir
from concourse.library_config import mlp as mlp_lib
from concourse.bass2jax import install_neuronx_cc_hook, _bass_exec_p, partition_id_tensor


F32 = mybir.dt.float32
I16 = mybir.dt.int16
NEG_SLOPE = 0.2
EPS = 1e-16


def build_launch(cfg):
    """cfg keys: shard, npad, n_ranges, range_size, heads, ch, rowf,
    nb [n_dblk, n_ranges], units [(r,u0,nbu)...], n_blocks_total, bpu,
    elu_out (bool), out_cols"""
    heads, ch = cfg['heads'], cfg['ch']
    HC = heads * ch
    shard, npad = cfg['shard'], cfg['npad']
    n_dblk = shard // 128
    n_chunks = npad // 128
    rowf = cfg['rowf']
    wcols = HC + 2 * heads
    nb = cfg['nb']
    units = cfg['units']
    bpu = cfg['bpu']
    ulen = bpu * 128
    n_units = len(units)
    n_blocks = cfg['n_blocks_total']
    n_ranges, range_size = cfg['n_ranges'], cfg['range_size']
    out_cols = cfg['out_cols']

    nc = bacc.Bacc(target_bir_lowering=False, num_swdge_queues=cfg.get('nq', 1))
    xT = nc.dram_tensor("xT", [128, npad], F32, kind="ExternalInput")
    wcat = nc.dram_tensor("wcat", [128, wcols], F32, kind="ExternalInput")
    krepd = nc.dram_tensor("krep", [128, heads], F32, kind="ExternalInput")
    brepd = nc.dram_tensor("brep", [128, HC], F32, kind="ExternalInput")
    iotad = nc.dram_tensor("iota", [128, 128], F32, kind="ExternalInput")
    identd = nc.dram_tensor("ident", [128, 128], F32, kind="ExternalInput")
    idx16d = nc.dram_tensor("idx16", [n_units, 128, ulen // 16], I16, kind="ExternalInput")
    streamd = nc.dram_tensor("stream", [n_units, 128, bpu, 2], F32, kind="ExternalInput")
    outd = nc.dram_tensor("out", [shard, out_cols], F32, kind="ExternalOutput")

    xs_tab = nc.dram_tensor("xs_tab", [npad, rowf], F32)
    ad_tab = nc.dram_tensor("ad_tab", [n_chunks, 128, heads], F32)

    nc.gpsimd.load_library(mlp_lib)

    with tile.TileContext(nc) as tc:
        # ---------------- phase 1: dense tables over ALL nodes (replicated)
        with (
            tc.tile_pool(name="p1c", bufs=1) as p1c,
            tc.tile_pool(name="p1x", bufs=4) as p1x,
            tc.tile_pool(name="p1s", bufs=4) as p1s,
            tc.tile_pool(name="p1ps", bufs=4, space="PSUM") as p1ps,
        ):
            wc_sb = p1c.tile([128, wcols], F32)
            nc.sync.dma_start(out=wc_sb[:], in_=wcat[:])
            for ci in range(n_chunks):
                xt = p1x.tile([128, 128], F32)
                nc.sync.dma_start(out=xt[:], in_=xT[:, ci * 128:(ci + 1) * 128])
                ps = p1ps.tile([128, wcols], F32)
                nc.tensor.matmul(ps[:], lhsT=xt[:], rhs=wc_sb[:], start=True, stop=True)
                st = p1s.tile([128, rowf], F32)
                nc.vector.tensor_copy(st[:, 0:wcols], ps[:])
                if rowf > wcols:
                    nc.vector.memset(st[:, wcols:rowf], 0)
                nc.sync.dma_start(out=xs_tab[ci * 128:(ci + 1) * 128, :],
                                  in_=st[:])
                nc.sync.dma_start(out=ad_tab[ci], in_=st[:, HC + heads:wcols])

        tc.strict_bb_all_engine_barrier()

        # ---------------- phase 2: edge pipeline
        with (
            tc.tile_pool(name="cst", bufs=1) as cst,
            tc.tile_pool(name="gx", bufs=6) as gx,
            tc.tile_pool(name="ix", bufs=4) as ixp,
            tc.tile_pool(name="stm", bufs=4) as stm,
            tc.tile_pool(name="stS", bufs=12) as stS,
            tc.tile_pool(name="stT", bufs=3) as stT,
            tc.tile_pool(name="zt", bufs=3) as ztp,
            tc.tile_pool(name="adp", bufs=6) as adp,
            tc.tile_pool(name="fin", bufs=2) as finp,
            tc.tile_pool(name="ups", bufs=4, space="PSUM") as upsp,
            tc.tile_pool(name="tps", bufs=2, space="PSUM") as tpsp,
            tc.tile_pool(name="zps", bufs=2, space="PSUM") as zpsp,
        ):
            iota_sb = cst.tile([128, 128], F32)
            nc.sync.dma_start(out=iota_sb[:], in_=iotad[:])
            ident_sb = cst.tile([128, 128], F32)
            nc.sync.dma_start(out=ident_sb[:], in_=identd[:])
            krep_sb = cst.tile([128, heads], F32)
            nc.sync.dma_start(out=krep_sb[:], in_=krepd[:])
            brep_sb = cst.tile([128, HC], F32)
            nc.sync.dma_start(out=brep_sb[:], in_=brepd[:])

            # mm2 totals per D for start/stop flags
            mm2_total = {Di: int(nb[Di].sum()) for Di in range(n_dblk)}
            mm2_done = {Di: 0 for Di in range(n_dblk)}
            ups_tiles = {}
            ad_tiles = {}

            # flat block index in stream tensor: r-major, then r-stream order
            rstream_base = [0] * n_ranges
            acc = 0
            for r in range(n_ranges):
                rstream_base[r] = acc
                acc += int(nb[:, r].sum())
            # D of each position in each r stream
            r_stream_D = []
            for r in range(n_ranges):
                lst = []
                for Di in range(n_dblk):
                    lst += [Di] * int(nb[Di, r])
                r_stream_D.append(lst)

            # process units sorted by (first D, r)
            order = sorted(range(n_units),
                           key=lambda ui: (r_stream_D[units[ui][0]][units[ui][1]],
                                           units[ui][0]))

            def finalize(Di, ups):
                sr = finp.tile([128, heads], F32, tag="sr")
                nc.vector.tensor_scalar_add(sr[:], ups[:, HC:HC + heads], EPS)
                rr = finp.tile([128, heads], F32, tag="rr")
                nc.vector.reciprocal(rr[:], sr[:])
                h0 = finp.tile([128, HC], F32, tag="h0")
                rb = bass.AP(tensor=rr[:].tensor, offset=rr[:].offset,
                             ap=[list(rr[:].ap[0]), list(rr[:].ap[1]), [0, ch]])
                nc.vector.tensor_tensor(
                    out=h0[:].rearrange("p (h c) -> p h c", h=heads),
                    in0=ups[:, 0:HC].rearrange("p (h c) -> p h c", h=heads),
                    in1=rb, op=mybir.AluOpType.mult)
                nc.vector.tensor_tensor(out=h0[:], in0=h0[:], in1=brep_sb[:],
                                        op=mybir.AluOpType.add)
                if cfg['elu_out']:
                    m0 = finp.tile([128, HC], F32, tag="m0")
                    nc.vector.tensor_scalar_min(m0[:], h0[:], 0.0)
                    e = finp.tile([128, HC], F32, tag="e")
                    nc.scalar.activation(out=e[:], in_=m0[:],
                                         func=mybir.ActivationFunctionType.Exp)
                    nc.vector.tensor_scalar_add(e[:], e[:], -1.0)
                    nc.vector.tensor_scalar_max(h0[:], h0[:], 0.0)
                    nc.vector.tensor_tensor(out=e[:], in0=e[:], in1=h0[:],
                                            op=mybir.AluOpType.add)
                    res = e
                else:
                    res = h0
                nc.sync.dma_start(out=outd[Di * 128:(Di + 1) * 128, :],
                                  in_=res[:, 0:out_cols])

            for ui in order:
                r, u0, nbu = units[ui]
                it = ixp.tile([128, ulen // 16], I16)
                nc.sync.dma_start(out=it[:], in_=idx16d[ui])
                gt = gx.tile([128, bpu, rowf], F32)
                nc.gpsimd.dma_gather(
                    out_ap=gt[:],
                    in_ap=xs_tab[r * range_size:(r + 1) * range_size, :],
                    idxs_ap=it[:], num_idxs=ulen, num_idxs_reg=ulen,
                    elem_size=rowf, queue_num=(ui % cfg.get('nq', 1)))

                zps = zpsp.tile([128, bpu, heads], F32)
                zsb = ztp.tile([128, bpu, heads], F32)
                blk_Ds = []
                sts = []
                ust = stm.tile([128, bpu, 2], F32)
                nc.sync.dma_start(out=ust[:], in_=streamd[ui])
                for s in range(nbu):
                    Di = r_stream_D[r][u0 + s]
                    blk_Ds.append(Di)
                    stt = ust[:, s, :]
                    sts.append(stt)
                    if Di not in ad_tiles:
                        adt = adp.tile([128, heads], F32)
                        nc.sync.dma_start(out=adt[:], in_=ad_tab[Di])
                        ad_tiles[Di] = adt
                    S = stS.tile([128, 128], F32)
                    nc.vector.tensor_tensor(
                        out=S[:], in0=stt[:, 0:1].to_broadcast([128, 128]),
                        in1=iota_sb[:], op=mybir.AluOpType.is_equal)
                    Tps = tpsp.tile([128, 128], F32)
                    nc.tensor.transpose(out=Tps[:], in_=S[:], identity=ident_sb[:])
                    Tsb = stT.tile([128, 128], F32)
                    nc.scalar.copy(out=Tsb[:], in_=Tps[:])
                    nc.tensor.matmul(zps[:, s, :], lhsT=Tsb[:], rhs=ad_tiles[Di][:],
                                     start=True, stop=True)
                    nc.vector.scalar_tensor_tensor(
                        out=zsb[:, s, :], in0=krep_sb[:], scalar=stt[:, 1:2],
                        in1=gt[:, s, HC:HC + heads],
                        op0=mybir.AluOpType.mult, op1=mybir.AluOpType.add)
                    nc.vector.tensor_tensor(out=zsb[:, s, :], in0=zsb[:, s, :],
                                            in1=zps[:, s, :], op=mybir.AluOpType.add)
                    sts.append(S)  # keep alive
                # batched lrelu + exp -> p into gt alpha cols
                zl = ztp.tile([128, bpu, heads], F32, tag="zl")
                nc.vector.tensor_scalar_mul(zl[:, 0:nbu, :], zsb[:, 0:nbu, :], NEG_SLOPE)
                nc.vector.tensor_tensor(out=zsb[:, 0:nbu, :], in0=zsb[:, 0:nbu, :],
                                        in1=zl[:, 0:nbu, :], op=mybir.AluOpType.max)
                nc.scalar.activation(out=gt[:, 0:nbu, HC:HC + heads],
                                     in_=zsb[:, 0:nbu, :],
                                     func=mybir.ActivationFunctionType.Exp)
                # per-block: scale msgs by p, then mm2
                for s in range(nbu):
                    Di = blk_Ds[s]
                    pb = gt[:, s, HC:HC + heads]
                    pbb = bass.AP(tensor=pb.tensor, offset=pb.offset,
                                  ap=[list(pb.ap[0]), list(pb.ap[1]), [0, ch]])
                    nc.vector.tensor_tensor(
                        out=gt[:, s, 0:HC].rearrange("p (h c) -> p h c", h=heads),
                        in0=gt[:, s, 0:HC].rearrange("p (h c) -> p h c", h=heads),
                        in1=pbb, op=mybir.AluOpType.mult)
                    if Di not in ups_tiles:
                        ups_tiles[Di] = upsp.tile([128, HC + heads], F32, name=f'ups{Di}', tag='ups')
                    first = (mm2_done[Di] == 0)
                    last = (mm2_done[Di] + 1 == mm2_total[Di])
                    # reuse this block's S tile as lhsT
                    Sl = sts[2 * s + 1]
                    nc.tensor.matmul(ups_tiles[Di][:], lhsT=Sl[:],
                                     rhs=gt[:, s, 0:HC + heads],
                                     start=first, stop=last, skip_group_check=True)
                    mm2_done[Di] += 1
                    if last:
                        finalize(Di, ups_tiles[Di])
                        del ups_tiles[Di]
                        del ad_tiles[Di]
    nc.compile()
    return nc


# ---------------------------------------------------------------- host plan

def make_plan(src, dst, ew, n_cores, shard, npad, n_ranges, bpu=8):
    range_size = npad // n_ranges
    n_dblk = shard // 128
    counts = np.zeros((n_cores, n_dblk, n_ranges), dtype=np.int64)
    core_of = dst // shard
    perm_pos = np.empty((n_cores, npad), np.int64)  # global node -> permuted pos
    for c in range(n_cores):
        base = c * shard
        pos = np.empty(npad, np.int64)
        pos[base:base + shard] = np.arange(shard)
        pos[:base] = shard + np.arange(base)
        pos[base + shard:] = np.arange(base + shard, npad)
        perm_pos[c] = pos

    edata = []
    for c in range(n_cores):
        m = core_of == c
        s_c, d_c, w_c = src[m], dst[m], ew[m]
        p_c = perm_pos[c][s_c]          # permuted src position
        base = c * shard
        D = (d_c - base) // 128
        R = p_c // range_size
        for Di_r in range(1):
            pass
        edata.append((p_c, d_c - base, w_c, D, R))
        for Di in range(n_dblk):
            mD = D == Di
            for r in range(n_ranges):
                counts[c, Di, r] = np.sum(mD & (R == r))
    nb = np.maximum(np.ceil(counts.max(axis=0) / 128).astype(np.int64),
                    0)  # [n_dblk, n_ranges]
    # ensure every D has at least one block total (coverage)
    for Di in range(n_dblk):
        if nb[Di].sum() == 0:
            nb[Di, 0] = 1

    r_stream_len = [int(nb[:, r].sum()) for r in range(n_ranges)]
    units = []
    for r in range(n_ranges):
        for u0 in range(0, r_stream_len[r], bpu):
            units.append((r, u0, min(bpu, r_stream_len[r] - u0)))
    n_blocks_total = sum(r_stream_len)
    ulen = bpu * 128

    # r-stream position of (Di, r, j)
    nb_cum = np.zeros((n_dblk + 1, n_ranges), np.int64)
    nb_cum[1:] = np.cumsum(nb, axis=0)

    per_core = []
    for c in range(n_cores):
        p_c, drel_all, w_c, D, R = edata[c]
        idx_all = np.zeros(n_blocks_total * 128, np.int64)
        dr_all = np.full(n_blocks_total * 128, -1.0, np.float32)
        ew_all = np.zeros(n_blocks_total * 128, np.float32)
        rbase = 0
        for r in range(n_ranges):
            for Di in range(n_dblk):
                sel = (D == Di) & (R == r)
                k = int(sel.sum())
                ss = p_c[sel] - r * range_size
                o = np.argsort(ss, kind='stable')
                start = (rbase + nb_cum[Di, r]) * 128
                idx_all[start:start + k] = ss[o]
                dr_all[start:start + k] = (drel_all[sel][o] - Di * 128).astype(np.float32)
                ew_all[start:start + k] = w_c[sel][o]
            rbase += r_stream_len[r]
        dr_b = dr_all.reshape(-1, 128)
        ew_b = ew_all.reshape(-1, 128)
        stream = np.zeros((len(units), 128, bpu, 2), np.float32)
        stream[:, :, :, 0] = -1.0
        # idx16 per unit
        idx16 = np.zeros((len(units), 128, ulen // 16), np.int16)
        rbase = 0
        ustarts = []
        for r in range(n_ranges):
            ustarts.append(rbase)
            rbase += r_stream_len[r]
        for ui, (r, u0, nbu) in enumerate(units):
            start = (ustarts[r] + u0) * 128
            idx = np.zeros(ulen, np.int64)
            idx[:nbu * 128] = idx_all[start:start + nbu * 128]
            for s in range(nbu):
                stream[ui, :, s, 0] = dr_b[ustarts[r] + u0 + s]
                stream[ui, :, s, 1] = ew_b[ustarts[r] + u0 + s]
            wrap = np.zeros((16, ulen // 16), np.int16)
            ii = np.arange(ulen)
            wrap[ii % 16, ii // 16] = idx
            idx16[ui] = np.tile(wrap, (8, 1))
        per_core.append(dict(idx16=idx16, stream=stream))

    return dict(nb=nb, units=units, n_blocks_total=n_blocks_total,
                bpu=bpu, per_core=per_core, perm_pos=perm_pos,
                n_ranges=n_ranges, range_size=range_size)


def blockdiag(att):
    """att [H, C] -> [H*C, H]"""
    H, C = att.shape
    A = np.zeros((H * C, H), np.float32)
    for h in range(H):
        A[h * C:(h + 1) * C, h] = att[h]
    return A




def gat_prepare(x, edge_index, edge_weight,
                W1, att_src1, att_dst1, att_edge1, We1, b1,
                W2, att_src2, att_dst2, att_edge2, We2, b2,
                n_cores=8, bpu=8):
    N, DIN = x.shape
    H, C1 = att_src1.shape
    C2 = att_src2.shape[1]
    shard = int(np.ceil(N / (128 * n_cores))) * 128
    npad = shard * n_cores
    n_ranges = int(np.ceil(npad / 32768 / 0.98))  # keep ranges < 32768
    n_ranges = max(n_ranges, 1)
    while npad % n_ranges or (npad // n_ranges) % 128:
        n_ranges += 1
    range_size = npad // n_ranges
    assert range_size <= 32767, (npad, n_ranges, range_size)

    src = np.asarray(edge_index[0], np.int64)
    dst = np.asarray(edge_index[1], np.int64)
    ew = np.asarray(edge_weight, np.float32)
    # self loops with mean edge weight
    si = np.arange(N, dtype=np.int64)
    src2 = np.concatenate([src, si])
    dst2 = np.concatenate([dst, si])
    ew2 = np.concatenate([ew, np.full(N, ew.mean(), np.float32)])

    plan = make_plan(src2, dst2, ew2, n_cores, shard, npad, n_ranges, bpu)

    # layer configs
    HC1 = H * C1
    rowf1 = 192 if HC1 + H > 128 else 128
    rowf2 = 128 if C2 + 1 <= 128 else 192
    k1 = np.array([np.dot(We1[0, h * C1:(h + 1) * C1], att_edge1[h]) for h in range(H)],
                  np.float32)
    k2 = np.array([np.dot(We2[0], att_edge2[0])], np.float32)
    wcat1 = np.concatenate([W1, W1 @ blockdiag(att_src1), W1 @ blockdiag(att_dst1)],
                           axis=1).astype(np.float32)  # [128, HC1+2H]
    wcat2 = np.concatenate([W2, W2 @ att_src2.T, W2 @ att_dst2.T],
                           axis=1).astype(np.float32)  # [HC1, C2+2]
    iota = np.tile(np.arange(128, dtype=np.float32), (128, 1))
    ident = np.eye(128, dtype=np.float32)

    cfg1 = dict(shard=shard, npad=npad, n_ranges=n_ranges, range_size=range_size,
                heads=H, ch=C1, rowf=rowf1, nb=plan['nb'], units=plan['units'],
                n_blocks_total=plan['n_blocks_total'], bpu=bpu,
                elu_out=True, out_cols=HC1, nq=4)
    cfg2 = dict(shard=shard, npad=npad, n_ranges=n_ranges, range_size=range_size,
                heads=1, ch=C2, rowf=rowf2, nb=plan['nb'], units=plan['units'],
                n_blocks_total=plan['n_blocks_total'], bpu=bpu,
                elu_out=False, out_cols=C2, nq=4)

    xpad = np.zeros((npad, DIN), np.float32)
    xpad[:N] = np.asarray(x, np.float32)

    meta = dict(N=N, npad=npad, shard=shard, n_cores=n_cores, plan=plan,
                H=H, C1=C1, C2=C2, HC1=HC1,
                wcat1=wcat1, wcat2=wcat2, k1=k1, k2=k2,
                b1=np.asarray(b1, np.float32), b2=np.asarray(b2, np.float32),
                iota=iota, ident=ident, cfg1=cfg1, cfg2=cfg2, xpad=xpad)
    return meta


def launch_inputs(meta, layer, dense_rows):
    """dense_rows: [npad, 128] f32 (x for L1, h for L2).
    Returns per-core in_maps."""
    plan = meta['plan']
    n_cores = meta['n_cores']
    H = meta['H'] if layer == 1 else 1
    HC = meta['HC1'] if layer == 1 else meta['C2']
    wcat = meta['wcat1'] if layer == 1 else meta['wcat2']
    k = meta['k1'] if layer == 1 else meta['k2']
    b = meta['b1'] if layer == 1 else meta['b2']
    in_maps = []
    for c in range(n_cores):
        perm = np.argsort(plan['perm_pos'][c], kind='stable')  # pos -> node
        xp = dense_rows[perm]                                   # permuted rows
        in_maps.append(dict(
            xT=np.ascontiguousarray(xp.T),
            wcat=wcat,
            krep=np.tile(k, (128, 1)),
            brep=np.tile(b, (128, 1)),
            iota=meta['iota'], ident=meta['ident'],
            idx16=plan['per_core'][c]['idx16'],
            stream=plan['per_core'][c]['stream'],
        ))
    return in_maps


def reference_np(x, edge_index, edge_weight, W1, att_src1, att_dst1, att_edge1,
                 We1, b1, W2, att_src2, att_dst2, att_edge2, We2, b2):
    """numpy port of the jax reference."""
    def gat_conv(x, src, dst, ew, W, a_src, a_dst, a_edge, We, b, heads, out_ch, concat):
        n = x.shape[0]
        xs = (x @ W).reshape(n, heads, out_ch)
        alpha_src = (xs * a_src[None]).sum(-1)
        alpha_dst = (xs * a_dst[None]).sum(-1)
        e = (ew[:, None] @ We).reshape(-1, heads, out_ch)
        alpha_edge = (e * a_edge[None]).sum(-1)
        alpha = alpha_src[src] + alpha_dst[dst] + alpha_edge
        alpha = np.where(alpha > 0, alpha, NEG_SLOPE * alpha)
        m = np.full((n, heads), -np.inf, np.float32)
        np.maximum.at(m, dst, alpha)
        ex = np.exp(alpha - m[dst])
        s = np.zeros((n, heads), np.float32)
        np.add.at(s, dst, ex)
        att = ex / (s[dst] + 1e-16)
        out = np.zeros((n, heads, out_ch), np.float32)
        np.add.at(out, dst, xs[src] * att[:, :, None])
        out = out.reshape(n, heads * out_ch) if concat else out.mean(axis=1)
        return out + b

    n = x.shape[0]
    src, dst = edge_index[0], edge_index[1]
    si = np.arange(n, dtype=src.dtype)
    src2 = np.concatenate([src, si]); dst2 = np.concatenate([dst, si])
    ew2 = np.concatenate([edge_weight, np.full(n, edge_weight.mean(), np.float32)])
    H, C1 = att_src1.shape
    C2 = att_src2.shape[1]
    h = gat_conv(x, src2, dst2, ew2, W1, att_src1, att_dst1, att_edge1, We1, b1, H, C1, True)
    h = np.where(h > 0, h, np.exp(np.minimum(h, 0)) - 1)
    return gat_conv(h, src2, dst2, ew2, W2, att_src2, att_dst2, att_edge2, We2, b2, 1, C2, False)




class SpmdRunner:
    def __init__(self, nc, n_cores=8):
        install_neuronx_cc_hook()
        self.nc = nc
        self.n_cores = n_cores
        partition_name = nc.partition_id_tensor.name if nc.partition_id_tensor else None
        in_names, out_names, out_avals, zero_outs = [], [], [], []
        for alloc in nc.m.functions[0].allocations:
            if not isinstance(alloc, mybir.MemoryLocationSet):
                continue
            name = alloc.memorylocations[0].name
            if alloc.kind == "ExternalInput":
                if name != partition_name:
                    in_names.append(name)
            elif alloc.kind == "ExternalOutput":
                out_names.append(name)
                shape = tuple(alloc.tensor_shape)
                dtype = mybir.dt.np(alloc.dtype)
                out_avals.append(jax.core.ShapedArray(shape, dtype))
                zero_outs.append(np.zeros(shape, dtype))
        self.in_names = list(in_names)
        self.out_names = out_names
        self.out_avals = out_avals
        self.zero_outs = zero_outs
        n_params = len(in_names)
        n_outs = len(out_avals)
        all_in_names = in_names + out_names
        if partition_name is not None:
            all_in_names.append(partition_name)

        def _body(*args):
            operands = list(args)
            if partition_name is not None:
                operands.append(partition_id_tensor())
            outs = _bass_exec_p.bind(
                *operands,
                out_avals=tuple(out_avals),
                in_names=tuple(all_in_names),
                out_names=tuple(out_names),
                lowering_input_output_aliases=(),
                sim_require_finite=False,
                sim_require_nnan=False,
                nc=nc,
            )
            return tuple(outs)

        devices = jax.devices()[:n_cores]
        self.mesh = Mesh(np.asarray(devices), ("core",))
        in_specs = (PartitionSpec("core"),) * (n_params + n_outs)
        out_specs = (PartitionSpec("core"),) * n_outs
        # no donation so we can re-run with the same buffers
        self.fn = jax.jit(
            shard_map(_body, mesh=self.mesh, in_specs=in_specs,
                      out_specs=out_specs, check_rep=False),
            keep_unused=True,
        )
        self._dev_args = None

    def stage(self, in_maps):
        """Upload per-core inputs to device once."""
        n = self.n_cores
        concat_in = [
            np.concatenate([np.asarray(in_maps[c][name]) for c in range(n)], axis=0)
            for name in self.in_names
        ]
        concat_zeros = [
            np.zeros((n * z.shape[0], *z.shape[1:]), z.dtype) for z in self.zero_outs
        ]
        self._dev_args = [jax.device_put(a) for a in concat_in + concat_zeros]

    def run(self):
        outs = self.fn(*self._dev_args)
        jax.block_until_ready(outs)
        return outs

    def results(self, outs):
        n = self.n_cores
        return [
            {name: np.asarray(outs[i]).reshape(n, *self.out_avals[i].shape)[c]
             for i, name in enumerate(self.out_names)}
            for c in range(n)
        ]

    def time_it(self, iters=5):
        self.run()  # warm
        ts = []
        for _ in range(iters):
            t0 = time.perf_counter()
            self.run()
            ts.append(time.perf_counter() - t0)
        return min(ts), ts


def build_floor(cfg, n_units, n_blocks):
    """Trivial kernel with identical I/O decls, for dispatch-floor timing."""
    heads, ch = cfg['heads'], cfg['ch']
    HC = heads * ch
    npad = cfg['npad']
    bpu = cfg['bpu']
    ulen = bpu * 128
    F32 = mybir.dt.float32
    nc = bacc.Bacc(target_bir_lowering=False)
    xT = nc.dram_tensor("xT", [128, npad], F32, kind="ExternalInput")
    nc.dram_tensor("wcat", [128, HC + 2 * heads], F32, kind="ExternalInput")
    nc.dram_tensor("krep", [128, heads], F32, kind="ExternalInput")
    nc.dram_tensor("brep", [128, HC], F32, kind="ExternalInput")
    nc.dram_tensor("iota", [128, 128], F32, kind="ExternalInput")
    nc.dram_tensor("ident", [128, 128], F32, kind="ExternalInput")
    nc.dram_tensor("idx16", [n_units, 128, ulen // 16], mybir.dt.int16,
                   kind="ExternalInput")
    nc.dram_tensor("stream", [n_units, 128, bpu, 2], F32, kind="ExternalInput")
    outd = nc.dram_tensor("out", [cfg['shard'], cfg['out_cols']], F32,
                          kind="ExternalOutput")
    with tile.TileContext(nc) as tc:
        with tc.tile_pool(name="s", bufs=2) as pool:
            t = pool.tile([128, 128], F32)
            nc.sync.dma_start(out=t[:], in_=xT[:, 0:128])
            for ci in range(cfg['shard'] // 128):
                nc.sync.dma_start(out=outd[ci * 128:(ci + 1) * 128, :],
                                  in_=t[:, 0:cfg['out_cols']])
    nc.compile()
    return nc


def kernel(**inputs):
    inputs = {k: np.asarray(v) for k, v in inputs.items()}
    x = inputs['x'].astype(np.float32)
    edge_index = inputs['edge_index'].astype(np.int64)
    ew = inputs['edge_weight'].astype(np.float32)
    args = (x, edge_index, ew,
            inputs['W1'].astype(np.float32), inputs['att_src1'].astype(np.float32),
            inputs['att_dst1'].astype(np.float32), inputs['att_edge1'].astype(np.float32),
            inputs['We1'].astype(np.float32), inputs['b1'].astype(np.float32),
            inputs['W2'].astype(np.float32), inputs['att_src2'].astype(np.float32),
            inputs['att_dst2'].astype(np.float32), inputs['att_edge2'].astype(np.float32),
            inputs['We2'].astype(np.float32), inputs['b2'].astype(np.float32))
    meta = gat_prepare(*args)
    nc1 = build_launch(meta['cfg1'])
    nc2 = build_launch(meta['cfg2'])
    N, shard, n_cores = meta['N'], meta['shard'], meta['n_cores']

    r1 = SpmdRunner(nc1, n_cores)
    in_maps1 = launch_inputs(meta, 1, meta['xpad'])
    r1.stage(in_maps1)
    res1 = r1.results(r1.run())
    hfull = np.concatenate([res1[c]['out'] for c in range(n_cores)], axis=0)
    hfull[N:] = 0.0

    r2 = SpmdRunner(nc2, n_cores)
    r2.stage(launch_inputs(meta, 2, hfull))
    res2 = r2.results(r2.run())
    out = np.concatenate([res2[c]['out'] for c in range(n_cores)], axis=0)[:N]

    floor_r = None
    try:
        ncf = build_floor(meta['cfg1'], len(meta['plan']['units']),
                          meta['plan']['n_blocks_total'])
        floor_r = SpmdRunner(ncf, n_cores)
        floor_r.stage(in_maps1)
        floor_r.run()
    except Exception:
        floor_r = None
    kernel._last = dict(meta=meta, r1=r1, r2=r2, nc1=nc1, nc2=nc2, floor=floor_r)
    return out.astype(np.float32)



# revision 2
# speedup vs baseline: 1.3714x; 1.3714x over previous
"""Self-contained 2-layer GAT kernel for Trainium2 (8 NeuronCores), v2.

Destination-sharded (each core owns its 12544-dst shard; no collectives).
Per layer the dense node table xs_tab[n, [msg | a_src | a_dst | pad]] is
computed in fp16 (phase 1, batched DMAs), then edges — bucketed host-side by
(dst-block of 128, src-range) into 128-edge blocks, bpu blocks per unit —
are processed with TWO SWDGE gathers per unit (source rows for messages +
alpha_src; destination rows for alpha_dst), one packed metadata blob DMA
(int16 gather indices, dst-rel row as fp16, k·edge_weight as fp16), one
batched one-hot build (is_equal vs iota), a short batched alpha pipeline
(add, lrelu, exp), and one PE matmul per 128-edge block that scatter-adds
p-scaled messages and the softmax denominator into the dst-block PSUM
accumulator. Finalize normalizes by the accumulated denominator, adds bias
(+ ELU for layer 1) and writes the dst shard. fp16 data / f32 accumulate.
"""
import sys
sys.path.insert(0, '/opt/trn_rl_repo')
import time
import numpy as np
import jax
from jax.sharding import Mesh, PartitionSpec
from jax.experimental.shard_map import shard_map

import concourse.bass as bass
import concourse.tile as tile
from concourse import bacc, mybir
from concourse.library_config import mlp as mlp_lib
from concourse.bass2jax import install_neuronx_cc_hook, _bass_exec_p, partition_id_tensor

F32 = mybir.dt.float32
F16 = mybir.dt.float16
I16 = mybir.dt.int16
NEG_SLOPE = 0.2
EPS = 1e-16
ALU = mybir.AluOpType


def _ap(src, dims):
    """Build an AP over src's tensor with explicit [stride, size] dims."""
    return bass.AP(tensor=src.tensor, offset=src.offset,
                   ap=[list(d) for d in dims])


def build_launch(cfg):
    """cfg keys: shard, npad, n_ranges, range_size, heads, ch, rowe,
    nb, units, n_blocks_total, bpu, elu_out, out_cols, out_f16, nq, scratch"""
    heads, ch = cfg['heads'], cfg['ch']
    HC = heads * ch
    shard, npad = cfg['shard'], cfg['npad']
    n_dblk = shard // 128
    n_chunks = npad // 128
    rowe = cfg['rowe']                  # fp16 elems per table row
    wcols = HC + 2 * heads
    assert wcols <= rowe
    dbase = 128 if HC + heads >= 128 else 0   # col window for the dst gather
    doff = HC + heads - dbase                 # a_dst col inside gd tile
    assert doff + heads <= 128
    nb = cfg['nb']
    units = cfg['units']
    bpu = cfg['bpu']
    ulen = bpu * 128
    wq = ulen // 16
    n_units = len(units)
    n_ranges, range_size = cfg['n_ranges'], cfg['range_size']
    out_cols = cfg['out_cols']
    OUTDT = F16 if cfg['out_f16'] else F32
    nq = cfg.get('nq', 4)
    BB = 2 * wq + bpu + bpu * heads     # blob int16 cols

    nc = bacc.Bacc(target_bir_lowering=False, num_swdge_queues=nq,
                   dynamic_dma_scratch_size=cfg.get('scratch', 16384))
    xT = nc.dram_tensor("xT", [128, npad], F16, kind="ExternalInput")
    wcat = nc.dram_tensor("wcat", [128, wcols], F16, kind="ExternalInput")
    brepd = nc.dram_tensor("brep", [128, HC], F32, kind="ExternalInput")
    iotad = nc.dram_tensor("iota", [128, 128], F16, kind="ExternalInput")
    blobd = nc.dram_tensor("blob", [n_units, 128, BB], I16, kind="ExternalInput")
    outd = nc.dram_tensor("out", [shard, out_cols], OUTDT, kind="ExternalOutput")

    xs_tab = nc.dram_tensor("xs_tab", [npad, rowe], F16)

    nc.gpsimd.load_library(mlp_lib)

    with tile.TileContext(nc) as tc:
        # ---------------- phase 1: dense node table (replicated over cores)
        CB = 16
        assert n_chunks % CB == 0, (n_chunks, CB)
        with (
            tc.tile_pool(name="p1c", bufs=1) as p1c,
            tc.tile_pool(name="p1x", bufs=3) as p1x,
            tc.tile_pool(name="p1s", bufs=3) as p1s,
            tc.tile_pool(name="p1ps", bufs=8, space="PSUM") as p1ps,
        ):
            wc_sb = p1c.tile([128, wcols], F16)
            nc.sync.dma_start(out=wc_sb[:], in_=wcat[:])
            for it in range(n_chunks // CB):
                xt = p1x.tile([128, CB, 128], F16)
                nc.sync.dma_start(
                    out=xt[:], in_=xT[:, it * CB * 128:(it + 1) * CB * 128])
                st = p1s.tile([128, CB, rowe], F16)
                if rowe > wcols:
                    nc.vector.memset(st[:, :, wcols:rowe], 0)
                for j in range(CB):
                    ps = p1ps.tile([128, wcols], F32)
                    nc.tensor.matmul(ps[:], lhsT=xt[:, j, :], rhs=wc_sb[:],
                                     start=True, stop=True)
                    nc.vector.tensor_copy(st[:, j, 0:wcols], ps[:])
                # rows it*CB*128 + j*128 + p  <-  st[p, j, :]
                dview = _ap(xs_tab[it * CB * 128:(it + 1) * CB * 128, :],
                            [[rowe, 128], [128 * rowe, CB], [1, rowe]])
                nc.sync.dma_start(out=dview, in_=st[:])

        tc.strict_bb_all_engine_barrier()

        # ---------------- phase 2: edge pipeline
        with (
            tc.tile_pool(name="cst", bufs=1) as cst,
            tc.tile_pool(name="blp", bufs=4) as blp,
            tc.tile_pool(name="gx", bufs=3) as gx,
            tc.tile_pool(name="gd", bufs=3) as gdp,
            tc.tile_pool(name="sal", bufs=3) as salp,
            tc.tile_pool(name="zp", bufs=3) as zp,
            tc.tile_pool(name="fin", bufs=2) as finp,
            tc.tile_pool(name="ups", bufs=6, space="PSUM") as upsp,
        ):
            iota_sb = cst.tile([128, 128], F16)
            nc.sync.dma_start(out=iota_sb[:], in_=iotad[:])
            brep_sb = cst.tile([128, HC], F32)
            nc.sync.dma_start(out=brep_sb[:], in_=brepd[:])

            mm2_total = {Di: int(nb[Di].sum()) for Di in range(n_dblk)}
            mm2_done = {Di: 0 for Di in range(n_dblk)}
            ups_tiles = {}

            # D of each position in each r stream (r-streams are D-major)
            r_stream_D = []
            for r in range(n_ranges):
                lst = []
                for Di in range(n_dblk):
                    lst += [Di] * int(nb[Di, r])
                r_stream_D.append(lst)

            order = sorted(range(n_units),
                           key=lambda ui: (r_stream_D[units[ui][0]][units[ui][1]],
                                           units[ui][0]))

            def finalize(Di, ups):
                sr = finp.tile([128, heads], F32, tag="sr")
                nc.vector.tensor_scalar_add(sr[:], ups[:, HC:HC + heads], EPS)
                rr = finp.tile([128, heads], F32, tag="rr")
                nc.vector.reciprocal(rr[:], sr[:])
                h0 = finp.tile([128, HC], F32, tag="h0")
                rb = _ap(rr[:], [rr[:].ap[0], rr[:].ap[1], [0, ch]])
                nc.vector.tensor_tensor(
                    out=h0[:].rearrange("p (h c) -> p h c", h=heads),
                    in0=ups[:, 0:HC].rearrange("p (h c) -> p h c", h=heads),
                    in1=rb, op=ALU.mult)
                nc.vector.tensor_tensor(out=h0[:], in0=h0[:], in1=brep_sb[:],
                                        op=ALU.add)
                res = finp.tile([128, out_cols], OUTDT, tag="res")
                if cfg['elu_out']:
                    m0 = finp.tile([128, HC], F32, tag="m0")
                    nc.vector.tensor_scalar_min(m0[:], h0[:], 0.0)
                    e = finp.tile([128, HC], F32, tag="e")
                    nc.scalar.activation(out=e[:], in_=m0[:],
                                         func=mybir.ActivationFunctionType.Exp)
                    nc.vector.tensor_scalar_add(e[:], e[:], -1.0)
                    nc.vector.tensor_scalar_max(h0[:], h0[:], 0.0)
                    nc.vector.tensor_tensor(out=res[:], in0=e[:], in1=h0[:],
                                            op=ALU.add)
                else:
                    nc.vector.tensor_copy(res[:], h0[:, 0:out_cols])
                nc.sync.dma_start(out=outd[Di * 128:(Di + 1) * 128, :],
                                  in_=res[:])

            no_vec = cfg.get('no_vec')
            no_mm = cfg.get('no_mm') or no_vec
            for ui in order:
                r, u0, nbu = units[ui]
                blob = blp.tile([128, BB], I16)
                nc.sync.dma_start(out=blob[:], in_=blobd[ui])
                idxS = blob[:, 0:wq]
                idxD = blob[:, wq:2 * wq]
                drv = blob[:, 2 * wq:2 * wq + bpu].bitcast(F16)
                kewv = blob[:, 2 * wq + bpu:2 * wq + bpu + bpu * heads].bitcast(F16)

                # Fixed queue per gather type: Pool-engine DMAs are emitted
                # strictly as (S, D) pairs, so Tile's 8-slot DMASW rotation
                # puts all S gathers on even slots and all D gathers on odd
                # slots — a constant queue per type keeps every DMASW sem
                # locked to a single SWDGE queue.
                gt = gx.tile([128, bpu, rowe], F16)
                nc.gpsimd.dma_gather(
                    out_ap=gt[:],
                    in_ap=xs_tab[r * range_size:(r + 1) * range_size, :],
                    idxs_ap=idxS, num_idxs=ulen, num_idxs_reg=ulen,
                    elem_size=rowe, queue_num=0)
                if not cfg.get('no_gd'):
                    gd = gdp.tile([128, bpu, 128], F16)
                    nc.gpsimd.dma_gather(
                        out_ap=gd[:],
                        in_ap=xs_tab[0:shard, dbase:dbase + 128],
                        idxs_ap=idxD, num_idxs=ulen, num_idxs_reg=ulen,
                        elem_size=128, elem_step=rowe, queue_num=0)

                if no_vec:
                    continue
                # one-hot S for all bpu blocks: S[e, s, d] = (dr[e,s] == d)
                S_all = salp.tile([128, bpu, 128], F16)
                dr3 = _ap(drv, [drv.ap[0], [1, bpu], [0, 128]])
                io3 = _ap(iota_sb[:], [iota_sb[:].ap[0], [0, bpu], [1, 128]])
                nc.vector.tensor_tensor(out=S_all[:], in0=dr3, in1=io3,
                                        op=ALU.is_equal)

                # alpha = kew + a_src[src] + a_dst[dst]; lrelu; exp -> p
                zsb = zp.tile([128, bpu, heads], F32, tag="zsb")
                kew3 = _ap(kewv, [kewv.ap[0], [heads, bpu], [1, heads]])
                nc.vector.tensor_tensor(out=zsb[:], in0=kew3,
                                        in1=gt[:, :, HC:HC + heads], op=ALU.add)
                if not cfg.get('no_gd'):
                    nc.vector.tensor_tensor(out=zsb[:], in0=zsb[:],
                                            in1=gd[:, :, doff:doff + heads],
                                            op=ALU.add)
                zl = zp.tile([128, bpu, heads], F32, tag="zl")
                nc.vector.tensor_scalar_mul(zl[:], zsb[:], NEG_SLOPE)
                nc.vector.tensor_tensor(out=zsb[:], in0=zsb[:], in1=zl[:],
                                        op=ALU.max)
                nc.scalar.activation(out=gt[:, :, HC:HC + heads], in_=zsb[:],
                                     func=mybir.ActivationFunctionType.Exp)

                # p-scale messages (per head), then scatter-add per block
                for h in range(heads):
                    pb = gt[:, :, HC + h:HC + h + 1]
                    pb3 = _ap(pb, [pb.ap[0], pb.ap[1], [0, ch]])
                    nc.vector.tensor_tensor(out=gt[:, :, h * ch:(h + 1) * ch],
                                            in0=gt[:, :, h * ch:(h + 1) * ch],
                                            in1=pb3, op=ALU.mult)
                if no_mm:
                    continue
                for s in range(nbu):
                    Di = r_stream_D[r][u0 + s]
                    if Di not in ups_tiles:
                        ups_tiles[Di] = upsp.tile([128, HC + heads], F32,
                                                  name=f'ups{Di}', tag='ups')
                    first = (mm2_done[Di] == 0)
                    last = (mm2_done[Di] + 1 == mm2_total[Di])
                    nc.tensor.matmul(ups_tiles[Di][:], lhsT=S_all[:, s, :],
                                     rhs=gt[:, s, 0:HC + heads],
                                     start=first, stop=last,
                                     skip_group_check=True)
                    mm2_done[Di] += 1
                    if last:
                        finalize(Di, ups_tiles[Di])
                        del ups_tiles[Di]
            if no_mm:
                t = finp.tile([128, out_cols], OUTDT)
                nc.vector.memset(t[:], 0)
                for Di in range(n_dblk):
                    nc.sync.dma_start(out=outd[Di * 128:(Di + 1) * 128, :],
                                      in_=t[:])
    nc.compile()
    return nc


# ---------------------------------------------------------------- host plan

def make_plan(src, dst, ew, n_cores, shard, npad, n_ranges, bpu, k1, k2):
    range_size = npad // n_ranges
    n_dblk = shard // 128
    counts = np.zeros((n_cores, n_dblk, n_ranges), dtype=np.int64)
    core_of = dst // shard
    perm_pos = np.empty((n_cores, npad), np.int64)  # global node -> permuted pos
    for c in range(n_cores):
        base = c * shard
        pos = np.empty(npad, np.int64)
        pos[base:base + shard] = np.arange(shard)
        pos[:base] = shard + np.arange(base)
        pos[base + shard:] = np.arange(base + shard, npad)
        perm_pos[c] = pos

    edata = []
    for c in range(n_cores):
        m = core_of == c
        s_c, d_c, w_c = src[m], dst[m], ew[m]
        p_c = perm_pos[c][s_c]          # permuted src position
        base = c * shard
        drel = d_c - base
        D = drel // 128
        R = p_c // range_size
        edata.append((p_c, drel, w_c, D, R))
        for Di in range(n_dblk):
            mD = D == Di
            for r in range(n_ranges):
                counts[c, Di, r] = np.sum(mD & (R == r))
    nb = np.ceil(counts.max(axis=0) / 128).astype(np.int64)  # [n_dblk, n_ranges]
    for Di in range(n_dblk):
        if nb[Di].sum() == 0:
            nb[Di, 0] = 1

    r_stream_len = [int(nb[:, r].sum()) for r in range(n_ranges)]
    units = []
    for r in range(n_ranges):
        for u0 in range(0, r_stream_len[r], bpu):
            units.append((r, u0, min(bpu, r_stream_len[r] - u0)))
    n_blocks_total = sum(r_stream_len)
    ulen = bpu * 128
    wq = ulen // 16

    nb_cum = np.zeros((n_dblk + 1, n_ranges), np.int64)
    nb_cum[1:] = np.cumsum(nb, axis=0)
    ustarts = []
    rbase = 0
    for r in range(n_ranges):
        ustarts.append(rbase)
        rbase += r_stream_len[r]

    heads1 = len(k1)
    heads2 = len(k2)
    BB1 = 2 * wq + bpu + bpu * heads1
    BB2 = 2 * wq + bpu + bpu * heads2

    per_core = []
    for c in range(n_cores):
        p_c, drel_all, w_c, D, R = edata[c]
        nslots = n_blocks_total * 128
        idxS_all = np.zeros(nslots, np.int16)
        idxD_all = np.zeros(nslots, np.int16)
        dr_all = np.full(nslots, -1.0, np.float16)
        ew_all = np.zeros(nslots, np.float32)
        rbase = 0
        for r in range(n_ranges):
            for Di in range(n_dblk):
                sel = (D == Di) & (R == r)
                k = int(sel.sum())
                if k:
                    ss = p_c[sel] - r * range_size
                    o = np.argsort(ss, kind='stable')
                    start = (rbase + nb_cum[Di, r]) * 128
                    idxS_all[start:start + k] = ss[o].astype(np.int16)
                    dsel = drel_all[sel][o]
                    idxD_all[start:start + k] = dsel.astype(np.int16)
                    dr_all[start:start + k] = (dsel - Di * 128).astype(np.float16)
                    ew_all[start:start + k] = w_c[sel][o]
            rbase += r_stream_len[r]

        idxS_b = idxS_all.reshape(-1, 128)
        idxD_b = idxD_all.reshape(-1, 128)
        dr_b = dr_all.reshape(-1, 128)
        ew_b = ew_all.reshape(-1, 128)

        def wrap16(vals):
            """[ulen] int16 -> [128, ulen//16] wrapped + replicated."""
            w = np.zeros((16, len(vals) // 16), np.int16)
            ii = np.arange(len(vals))
            w[ii % 16, ii // 16] = vals
            return np.tile(w, (8, 1))

        blob1 = np.zeros((len(units), 128, BB1), np.int16)
        blob2 = np.zeros((len(units), 128, BB2), np.int16)
        for ui, (r, u0, nbu) in enumerate(units):
            b0 = ustarts[r] + u0
            iS = np.zeros(ulen, np.int16)
            iD = np.zeros(ulen, np.int16)
            iS[:nbu * 128] = idxS_b[b0:b0 + nbu].ravel()
            iD[:nbu * 128] = idxD_b[b0:b0 + nbu].ravel()
            for bl in (blob1, blob2):
                bl[ui, :, 0:wq] = wrap16(iS)
                bl[ui, :, wq:2 * wq] = wrap16(iD)
            dr_u = np.full((128, bpu), -1.0, np.float16)
            dr_u[:, :nbu] = dr_b[b0:b0 + nbu].T
            ew_u = np.zeros((128, bpu), np.float32)
            ew_u[:, :nbu] = ew_b[b0:b0 + nbu].T
            for bl, kk, hh in ((blob1, k1, heads1), (blob2, k2, heads2)):
                bl[ui, :, 2 * wq:2 * wq + bpu] = dr_u.view(np.int16)
                kew = (ew_u[:, :, None] * kk[None, None, :]).astype(np.float16)
                bl[ui, :, 2 * wq + bpu:2 * wq + bpu + bpu * hh] = \
                    kew.reshape(128, bpu * hh).view(np.int16)
        per_core.append(dict(blob1=blob1, blob2=blob2))

    return dict(nb=nb, units=units, n_blocks_total=n_blocks_total,
                bpu=bpu, per_core=per_core, perm_pos=perm_pos,
                n_ranges=n_ranges, range_size=range_size)


def blockdiag(att):
    """att [H, C] -> [H*C, H]"""
    H, C = att.shape
    A = np.zeros((H * C, H), np.float32)
    for h in range(H):
        A[h * C:(h + 1) * C, h] = att[h]
    return A


def gat_prepare(x, edge_index, edge_weight,
                W1, att_src1, att_dst1, att_edge1, We1, b1,
                W2, att_src2, att_dst2, att_edge2, We2, b2,
                n_cores=8, bpu=8):
    N, DIN = x.shape
    H, C1 = att_src1.shape
    C2 = att_src2.shape[1]
    shard = int(np.ceil(N / (128 * n_cores))) * 128
    npad = shard * n_cores
    n_ranges = 1
    while npad // n_ranges > 32767 or npad % n_ranges or (npad // n_ranges) % 128:
        n_ranges += 1
    range_size = npad // n_ranges

    src = np.asarray(edge_index[0], np.int64)
    dst = np.asarray(edge_index[1], np.int64)
    ew = np.asarray(edge_weight, np.float32)
    si = np.arange(N, dtype=np.int64)
    src2 = np.concatenate([src, si])
    dst2 = np.concatenate([dst, si])
    ew2 = np.concatenate([ew, np.full(N, ew.mean(), np.float32)])

    HC1 = H * C1
    k1 = np.array([np.dot(We1[0, h * C1:(h + 1) * C1], att_edge1[h])
                   for h in range(H)], np.float32)
    k2 = np.array([np.dot(We2[0], att_edge2[0])], np.float32)

    plan = make_plan(src2, dst2, ew2, n_cores, shard, npad, n_ranges, bpu,
                     k1, k2)

    rowe1 = 256 if HC1 + 2 * H > 128 else 128
    rowe2 = 256 if C2 + 2 > 128 else 128
    wcat1 = np.concatenate([W1, W1 @ blockdiag(att_src1), W1 @ blockdiag(att_dst1)],
                           axis=1).astype(np.float16)
    wcat2 = np.concatenate([W2, W2 @ att_src2.T, W2 @ att_dst2.T],
                           axis=1).astype(np.float16)
    iota = np.tile(np.arange(128, dtype=np.float16), (128, 1))

    common = dict(shard=shard, npad=npad, n_ranges=n_ranges,
                  range_size=range_size, nb=plan['nb'], units=plan['units'],
                  n_blocks_total=plan['n_blocks_total'], bpu=bpu, nq=1,
                  scratch=16384)
    cfg1 = dict(common, heads=H, ch=C1, rowe=rowe1, elu_out=True,
                out_cols=HC1, out_f16=True)
    cfg2 = dict(common, heads=1, ch=C2, rowe=rowe2, elu_out=False,
                out_cols=C2, out_f16=False)

    xpad = np.zeros((npad, DIN), np.float16)
    xpad[:N] = np.asarray(x, np.float16)

    meta = dict(N=N, npad=npad, shard=shard, n_cores=n_cores, plan=plan,
                H=H, C1=C1, C2=C2, HC1=HC1,
                wcat1=wcat1, wcat2=wcat2, k1=k1, k2=k2,
                b1=np.asarray(b1, np.float32), b2=np.asarray(b2, np.float32),
                iota=iota, cfg1=cfg1, cfg2=cfg2, xpad=xpad)
    return meta


def launch_inputs(meta, layer, dense_rows):
    """dense_rows: [npad, DIN] fp16 (x for L1, h for L2)."""
    plan = meta['plan']
    n_cores = meta['n_cores']
    HC = meta['HC1'] if layer == 1 else meta['C2']
    wcat = meta['wcat1'] if layer == 1 else meta['wcat2']
    b = meta['b1'] if layer == 1 else meta['b2']
    in_maps = []
    for c in range(n_cores):
        perm = np.argsort(plan['perm_pos'][c], kind='stable')  # pos -> node
        xp = dense_rows[perm]
        in_maps.append(dict(
            xT=np.ascontiguousarray(xp.T),
            wcat=wcat,
            brep=np.tile(b, (128, 1)).astype(np.float32),
            iota=meta['iota'],
            blob=plan['per_core'][c][f'blob{layer}'],
        ))
    return in_maps


class SpmdRunner:
    def __init__(self, nc, n_cores=8):
        install_neuronx_cc_hook()
        self.nc = nc
        self.n_cores = n_cores
        partition_name = nc.partition_id_tensor.name if nc.partition_id_tensor else None
        in_names, out_names, out_avals, zero_outs = [], [], [], []
        for alloc in nc.m.functions[0].allocations:
            if not isinstance(alloc, mybir.MemoryLocationSet):
                continue
            name = alloc.memorylocations[0].name
            if alloc.kind == "ExternalInput":
                if name != partition_name:
                    in_names.append(name)
            elif alloc.kind == "ExternalOutput":
                out_names.append(name)
                shape = tuple(alloc.tensor_shape)
                dtype = mybir.dt.np(alloc.dtype)
                out_avals.append(jax.core.ShapedArray(shape, dtype))
                zero_outs.append(np.zeros(shape, dtype))
        self.in_names = list(in_names)
        self.out_names = out_names
        self.out_avals = out_avals
        self.zero_outs = zero_outs
        n_params = len(in_names)
        n_outs = len(out_avals)
        all_in_names = in_names + out_names
        if partition_name is not None:
            all_in_names.append(partition_name)

        def _body(*args):
            operands = list(args)
            if partition_name is not None:
                operands.append(partition_id_tensor())
            outs = _bass_exec_p.bind(
                *operands,
                out_avals=tuple(out_avals),
                in_names=tuple(all_in_names),
                out_names=tuple(out_names),
                lowering_input_output_aliases=(),
                sim_require_finite=False,
                sim_require_nnan=False,
                nc=nc,
            )
            return tuple(outs)

        devices = jax.devices()[:n_cores]
        self.mesh = Mesh(np.asarray(devices), ("core",))
        in_specs = (PartitionSpec("core"),) * (n_params + n_outs)
        out_specs = (PartitionSpec("core"),) * n_outs
        self.fn = jax.jit(
            shard_map(_body, mesh=self.mesh, in_specs=in_specs,
                      out_specs=out_specs, check_rep=False),
            keep_unused=True,
        )
        self._dev_args = None

    def stage(self, in_maps):
        n = self.n_cores
        concat_in = [
            np.concatenate([np.asarray(in_maps[c][name]) for c in range(n)], axis=0)
            for name in self.in_names
        ]
        concat_zeros = [
            np.zeros((n * z.shape[0], *z.shape[1:]), z.dtype) for z in self.zero_outs
        ]
        self._dev_args = [jax.device_put(a) for a in concat_in + concat_zeros]

    def run(self):
        outs = self.fn(*self._dev_args)
        jax.block_until_ready(outs)
        return outs

    def results(self, outs):
        n = self.n_cores
        return [
            {name: np.asarray(outs[i]).reshape(n, *self.out_avals[i].shape)[c]
             for i, name in enumerate(self.out_names)}
            for c in range(n)
        ]

    def time_it(self, iters=5):
        self.run()
        ts = []
        for _ in range(iters):
            t0 = time.perf_counter()
            self.run()
            ts.append(time.perf_counter() - t0)
        return min(ts), ts


def build_floor(cfg, n_units):
    """Trivial kernel with identical I/O decls, for dispatch-floor timing."""
    heads = cfg['heads']
    HC = heads * cfg['ch']
    npad = cfg['npad']
    bpu = cfg['bpu']
    ulen = bpu * 128
    wq = ulen // 16
    BB = 2 * wq + bpu + bpu * heads
    OUTDT = F16 if cfg['out_f16'] else F32
    nc = bacc.Bacc(target_bir_lowering=False)
    xT = nc.dram_tensor("xT", [128, npad], F16, kind="ExternalInput")
    nc.dram_tensor("wcat", [128, HC + 2 * heads], F16, kind="ExternalInput")
    nc.dram_tensor("brep", [128, HC], F32, kind="ExternalInput")
    nc.dram_tensor("iota", [128, 128], F16, kind="ExternalInput")
    nc.dram_tensor("blob", [n_units, 128, BB], I16, kind="ExternalInput")
    outd = nc.dram_tensor("out", [cfg['shard'], cfg['out_cols']], OUTDT,
                          kind="ExternalOutput")
    with tile.TileContext(nc) as tc:
        with tc.tile_pool(name="s", bufs=2) as pool:
            t0 = pool.tile([128, 128], F16)
            nc.sync.dma_start(out=t0[:], in_=xT[:, 0:128])
            t = pool.tile([128, cfg['out_cols']], OUTDT)
            nc.vector.memset(t[:], 0)
            for ci in range(cfg['shard'] // 128):
                nc.sync.dma_start(out=outd[ci * 128:(ci + 1) * 128, :], in_=t[:])
    nc.compile()
    return nc


def kernel(**inputs):
    inputs = {k: np.asarray(v) for k, v in inputs.items()}
    x = inputs['x'].astype(np.float32)
    edge_index = inputs['edge_index'].astype(np.int64)
    ew = inputs['edge_weight'].astype(np.float32)
    meta = gat_prepare(
        x, edge_index, ew,
        inputs['W1'].astype(np.float32), inputs['att_src1'].astype(np.float32),
        inputs['att_dst1'].astype(np.float32), inputs['att_edge1'].astype(np.float32),
        inputs['We1'].astype(np.float32), inputs['b1'].astype(np.float32),
        inputs['W2'].astype(np.float32), inputs['att_src2'].astype(np.float32),
        inputs['att_dst2'].astype(np.float32), inputs['att_edge2'].astype(np.float32),
        inputs['We2'].astype(np.float32), inputs['b2'].astype(np.float32))
    nc1 = build_launch(meta['cfg1'])
    nc2 = build_launch(meta['cfg2'])
    N, shard, n_cores = meta['N'], meta['shard'], meta['n_cores']

    r1 = SpmdRunner(nc1, n_cores)
    r1.stage(launch_inputs(meta, 1, meta['xpad']))
    res1 = r1.results(r1.run())
    hfull = np.concatenate([res1[c]['out'] for c in range(n_cores)], axis=0)
    hfull[N:] = 0

    r2 = SpmdRunner(nc2, n_cores)
    r2.stage(launch_inputs(meta, 2, hfull))
    res2 = r2.results(r2.run())
    out = np.concatenate([res2[c]['out'] for c in range(n_cores)], axis=0)[:N]

    floor_r = None
    try:
        ncf = build_floor(meta['cfg1'], len(meta['plan']['units']))
        floor_r = SpmdRunner(ncf, n_cores)
        floor_r.stage(launch_inputs(meta, 1, meta['xpad']))
        floor_r.run()
    except Exception:
        floor_r = None
    kernel._last = dict(meta=meta, r1=r1, r2=r2, nc1=nc1, nc2=nc2, floor=floor_r)
    return out.astype(np.float32)


# revision 4
# speedup vs baseline: 1.4556x; 1.0614x over previous
"""Self-contained 2-layer GAT kernel for Trainium2 (8 NeuronCores), v2.

Destination-sharded (each core owns its 12544-dst shard; no collectives).
Per layer the dense node table xs_tab[n, [msg | a_src | a_dst | pad]] is
computed in fp16 (phase 1, batched DMAs), then edges — bucketed host-side by
(dst-block of 128, src-range) into 128-edge blocks, bpu blocks per unit —
are processed with TWO SWDGE gathers per unit (source rows for messages +
alpha_src; destination rows for alpha_dst), one packed metadata blob DMA
(int16 gather indices, dst-rel row as fp16, k·edge_weight as fp16), one
batched one-hot build (is_equal vs iota), a short batched alpha pipeline
(add, lrelu, exp), and one PE matmul per 128-edge block that scatter-adds
p-scaled messages and the softmax denominator into the dst-block PSUM
accumulator. Finalize normalizes by the accumulated denominator, adds bias
(+ ELU for layer 1) and writes the dst shard. fp16 data / f32 accumulate.
"""
import sys
sys.path.insert(0, '/opt/trn_rl_repo')
import time
import numpy as np
import jax
from jax.sharding import Mesh, PartitionSpec
from jax.experimental.shard_map import shard_map

import concourse.bass as bass
import concourse.tile as tile
from concourse import bacc, mybir
from concourse.library_config import mlp as mlp_lib
from concourse.bass2jax import install_neuronx_cc_hook, _bass_exec_p, partition_id_tensor

F32 = mybir.dt.float32
F16 = mybir.dt.float16
I16 = mybir.dt.int16
NEG_SLOPE = 0.2
EPS = 1e-16
ALU = mybir.AluOpType


def _ap(src, dims):
    """Build an AP over src's tensor with explicit [stride, size] dims."""
    return bass.AP(tensor=src.tensor, offset=src.offset,
                   ap=[list(d) for d in dims])


def build_launch(cfg):
    """cfg keys: shard, npad, n_ranges, range_size, heads, ch, rowe,
    nb, units, n_blocks_total, bpu, elu_out, out_cols, out_f16, nq, scratch"""
    heads, ch = cfg['heads'], cfg['ch']
    HC = heads * ch
    shard, npad = cfg['shard'], cfg['npad']
    n_dblk = shard // 128
    n_chunks = npad // 128
    rowe = cfg['rowe']                  # fp16 elems per table row
    wcols = HC + 2 * heads
    assert wcols <= rowe
    dbase = 128 if HC + heads >= 128 else 0   # col window for the dst gather
    doff = HC + heads - dbase                 # a_dst col inside gd tile
    assert doff + heads <= 128
    nb = cfg['nb']
    units = cfg['units']
    bpu = cfg['bpu']
    ulen = bpu * 128
    wq = ulen // 16
    n_units = len(units)
    n_ranges, range_size = cfg['n_ranges'], cfg['range_size']
    out_cols = cfg['out_cols']
    OUTDT = F16 if cfg['out_f16'] else F32
    nq = cfg.get('nq', 4)
    BB = 2 * wq + bpu + bpu * heads     # blob int16 cols

    admode = cfg.get('admode', 'mm')
    nc = bacc.Bacc(target_bir_lowering=False, num_swdge_queues=nq,
                   dynamic_dma_scratch_size=cfg.get('scratch', 16384),
                   detect_race_conditions=not cfg.get('prep'))
    xT = nc.dram_tensor("xT", [128, npad], F16, kind="ExternalInput")
    wcat = nc.dram_tensor("wcat", [128, wcols], F16, kind="ExternalInput")
    brepd = nc.dram_tensor("brep", [128, HC], F32, kind="ExternalInput")
    iotad = nc.dram_tensor("iota", [128, 128], F16, kind="ExternalInput")
    blobd = nc.dram_tensor("blob", [n_units, 128, BB], I16, kind="ExternalInput")
    if admode == 'mm':
        dtd = nc.dram_tensor("dt", [n_units, 1, ulen], F16, kind="ExternalInput")
        iotacd = nc.dram_tensor("iotac", [128, 8], F16, kind="ExternalInput")
    outd = nc.dram_tensor("out", [shard, out_cols], OUTDT, kind="ExternalOutput")

    xs_tab = nc.dram_tensor("xs_tab", [npad, rowe], F16)

    nc.gpsimd.load_library(mlp_lib)

    with tile.TileContext(nc) as tc:
        # ---------------- phase 1: dense node table (replicated over cores)
        CB = 16
        assert n_chunks % CB == 0, (n_chunks, CB)
        with (
            tc.tile_pool(name="p1c", bufs=1) as p1c,
            tc.tile_pool(name="p1x", bufs=3) as p1x,
            tc.tile_pool(name="p1s", bufs=3) as p1s,
            tc.tile_pool(name="p1ps", bufs=8, space="PSUM") as p1ps,
        ):
            wc_sb = p1c.tile([128, wcols], F16)
            nc.sync.dma_start(out=wc_sb[:], in_=wcat[:])
            for it in range(n_chunks // CB):
                xt = p1x.tile([128, CB, 128], F16)
                nc.sync.dma_start(
                    out=xt[:], in_=xT[:, it * CB * 128:(it + 1) * CB * 128])
                st = p1s.tile([128, CB, rowe], F16)
                if rowe > wcols:
                    nc.vector.memset(st[:, :, wcols:rowe], 0)
                for j in range(CB):
                    ps = p1ps.tile([128, wcols], F32)
                    nc.tensor.matmul(ps[:], lhsT=xt[:, j, :], rhs=wc_sb[:],
                                     start=True, stop=True)
                    nc.vector.tensor_copy(st[:, j, 0:wcols], ps[:])
                # rows it*CB*128 + j*128 + p  <-  st[p, j, :]
                dview = _ap(xs_tab[it * CB * 128:(it + 1) * CB * 128, :],
                            [[rowe, 128], [128 * rowe, CB], [1, rowe]])
                nc.sync.dma_start(out=dview, in_=st[:])

        tc.strict_bb_all_engine_barrier()

        # max concurrently-open dst-block accumulators under the chosen order
        _r_stream_D = []
        for _r in range(n_ranges):
            _l = []
            for _Di in range(n_dblk):
                _l += [_Di] * int(nb[_Di, _r])
            _r_stream_D.append(_l)
        _order = sorted(range(n_units),
                        key=lambda ui: (_r_stream_D[units[ui][0]][units[ui][1]],
                                        units[ui][0]))
        _tot = {d: int(nb[d].sum()) for d in range(n_dblk)}
        _done = {d: 0 for d in range(n_dblk)}
        _open, _max_open = set(), 1
        for _ui in _order:
            _r, _u0, _nbu = units[_ui]
            for _s in range(_nbu):
                _d = _r_stream_D[_r][_u0 + _s]
                _open.add(_d)
                _done[_d] += 1
                if _done[_d] == _tot[_d]:
                    _open.discard(_d)
                _max_open = max(_max_open, len(_open))
        ups_bufs = max(2, _max_open)
        assert ups_bufs + 3 <= 8, (ups_bufs, "PSUM banks over budget")

        # ---------------- phase 2: edge pipeline
        with (
            tc.tile_pool(name="cst", bufs=1) as cst,
            tc.tile_pool(name="blp", bufs=4) as blp,
            tc.tile_pool(name="gx", bufs=3) as gx,
            tc.tile_pool(name="gd", bufs=3) as gdp,
            tc.tile_pool(name="sal", bufs=3) as salp,
            tc.tile_pool(name="dtp", bufs=3) as dtp,
            tc.tile_pool(name="stp", bufs=3) as stp,
            tc.tile_pool(name="zp", bufs=3) as zp,
            tc.tile_pool(name="fin", bufs=2) as finp,
            tc.tile_pool(name="ups", bufs=ups_bufs, space="PSUM") as upsp,
            tc.tile_pool(name="dps", bufs=2, space="PSUM") as dpsp,
            tc.tile_pool(name="zps", bufs=1, space="PSUM") as zpsp,
        ):
            iota_sb = cst.tile([128, 128], F16)
            nc.sync.dma_start(out=iota_sb[:], in_=iotad[:])
            brep_sb = cst.tile([128, HC], F32)
            nc.sync.dma_start(out=brep_sb[:], in_=brepd[:])
            if admode == 'mm':
                iotac_sb = cst.tile([128, 8], F16)
                nc.sync.dma_start(out=iotac_sb[:], in_=iotacd[:])
                ones_sb = cst.tile([1, 128], F16)
                nc.vector.memset(ones_sb[:], 1.0)
                # a_dst for the core's own dst shard, [q, Di, h] from xs_tab
                ad_all = cst.tile([128, n_dblk, heads], F16)
                adview = _ap(xs_tab[0:shard, HC + heads:HC + 2 * heads],
                             [[rowe, 128], [128 * rowe, n_dblk], [1, heads]])
                nc.sync.dma_start(out=ad_all[:], in_=adview)

            mm2_total = {Di: int(nb[Di].sum()) for Di in range(n_dblk)}
            mm2_done = {Di: 0 for Di in range(n_dblk)}
            ups_tiles = {}

            # D of each position in each r stream (r-streams are D-major)
            r_stream_D = []
            for r in range(n_ranges):
                lst = []
                for Di in range(n_dblk):
                    lst += [Di] * int(nb[Di, r])
                r_stream_D.append(lst)

            order = sorted(range(n_units),
                           key=lambda ui: (r_stream_D[units[ui][0]][units[ui][1]],
                                           units[ui][0]))

            def finalize(Di, ups):
                sr = finp.tile([128, heads], F32, tag="sr")
                nc.vector.tensor_scalar_add(sr[:], ups[:, HC:HC + heads], EPS)
                rr = finp.tile([128, heads], F32, tag="rr")
                nc.vector.reciprocal(rr[:], sr[:])
                h0 = finp.tile([128, HC], F32, tag="h0")
                rb = _ap(rr[:], [rr[:].ap[0], rr[:].ap[1], [0, ch]])
                nc.vector.tensor_tensor(
                    out=h0[:].rearrange("p (h c) -> p h c", h=heads),
                    in0=ups[:, 0:HC].rearrange("p (h c) -> p h c", h=heads),
                    in1=rb, op=ALU.mult)
                nc.vector.tensor_tensor(out=h0[:], in0=h0[:], in1=brep_sb[:],
                                        op=ALU.add)
                res = finp.tile([128, out_cols], OUTDT, tag="res")
                if cfg['elu_out']:
                    m0 = finp.tile([128, HC], F32, tag="m0")
                    nc.vector.tensor_scalar_min(m0[:], h0[:], 0.0)
                    e = finp.tile([128, HC], F32, tag="e")
                    nc.scalar.activation(out=e[:], in_=m0[:],
                                         func=mybir.ActivationFunctionType.Exp)
                    nc.vector.tensor_scalar_add(e[:], e[:], -1.0)
                    nc.vector.tensor_scalar_max(h0[:], h0[:], 0.0)
                    nc.vector.tensor_tensor(out=res[:], in0=e[:], in1=h0[:],
                                            op=ALU.add)
                else:
                    nc.vector.tensor_copy(res[:], h0[:, 0:out_cols])
                nc.sync.dma_start(out=outd[Di * 128:(Di + 1) * 128, :],
                                  in_=res[:])

            no_vec = cfg.get('no_vec')
            no_mm = cfg.get('no_mm') or no_vec
            prep = cfg.get('prep')
            pd = cfg.get('pd', 1)
            dma_sem = nc.alloc_semaphore("gsem") if prep else None
            if cfg.get('p1_only'):
                order = []
                no_mm = True
            for k, ui in enumerate(order):
                r, u0, nbu = units[ui]
                blob = blp.tile([128, BB], I16)
                nc.sync.dma_start(out=blob[:], in_=blobd[ui])
                idxS = blob[:, 0:wq]
                idxD = blob[:, wq:2 * wq]
                drv = blob[:, 2 * wq:2 * wq + bpu].bitcast(F16)
                kewv = blob[:, 2 * wq + bpu:2 * wq + bpu + bpu * heads].bitcast(F16)

                # Fixed queue per gather type: Pool-engine DMAs are emitted
                # strictly as (S, D) pairs, so Tile's 8-slot DMASW rotation
                # puts all S gathers on even slots and all D gathers on odd
                # slots — a constant queue per type keeps every DMASW sem
                # locked to a single SWDGE queue.
                gt = gx.tile([128, bpu, rowe], F16)
                gd = gdp.tile([128, bpu, 128], F16) if admode == 'gather' else None
                if admode == 'mm':
                    dt_sb = dtp.tile([1, ulen], F16)
                    nc.sync.dma_start(out=dt_sb[:], in_=dtd[ui])
                if prep:
                    # prepare_only pipelining: desc-gen of unit k overlaps the
                    # in-flight transfers of units k-pd+1..k-1; the Pool-side
                    # wait bounds the SWDGE ring to pd units' descriptors.
                    if k >= pd:
                        nc.gpsimd.wait_ge(dma_sem, 32 * (k - pd + 1))
                    nc.gpsimd.dma_gather(
                        out_ap=gt[:],
                        in_ap=xs_tab[r * range_size:(r + 1) * range_size, :],
                        idxs_ap=idxS, num_idxs=ulen, num_idxs_reg=ulen,
                        elem_size=rowe, queue_num=0,
                        prepare_only=True, sem=dma_sem)
                    nc.gpsimd.dma_gather(
                        out_ap=gd[:],
                        in_ap=xs_tab[0:shard, dbase:dbase + 128],
                        idxs_ap=idxD, num_idxs=ulen, num_idxs_reg=ulen,
                        elem_size=128, elem_step=rowe, queue_num=0,
                        prepare_only=True, sem=dma_sem)
                    nc.gpsimd.trigger_dma(count=None)
                    if not no_vec:
                        nc.vector.wait_ge(dma_sem, 32 * (k + 1))
                else:
                    nc.gpsimd.dma_gather(
                        out_ap=gt[:],
                        in_ap=xs_tab[r * range_size:(r + 1) * range_size, :],
                        idxs_ap=idxS, num_idxs=ulen, num_idxs_reg=ulen,
                        elem_size=rowe, queue_num=0)
                    if admode == 'gather' and not cfg.get('no_gd'):
                        nc.gpsimd.dma_gather(
                            out_ap=gd[:],
                            in_ap=xs_tab[0:shard, dbase:dbase + 128],
                            idxs_ap=idxD, num_idxs=ulen, num_idxs_reg=ulen,
                            elem_size=128, elem_step=rowe, queue_num=0)

                if no_vec:
                    continue
                # one-hot S for all bpu blocks: S[e, s, d] = (dr[e,s] == d)
                S_all = salp.tile([128, bpu, 128], F16)
                dr3 = _ap(drv, [drv.ap[0], [1, bpu], [0, 128]])
                io3 = _ap(iota_sb[:], [iota_sb[:].ap[0], [0, bpu], [1, 128]])
                nc.vector.tensor_tensor(out=S_all[:], in0=dr3, in1=io3,
                                        op=ALU.is_equal)

                if admode == 'mm':
                    # S^T one-hot [d, s, q] built from the partition-broadcast
                    # of dr (PE outer product with a ones column), then
                    # alpha_dst per edge via one tiny matmul per block.
                    # 512-col chunks: a matmul output must stay in one PSUM bank.
                    ST_all = stp.tile([128, bpu, 128], F16)
                    for c0 in range(0, ulen, 512):
                        cw = min(512, ulen - c0)
                        dps = dpsp.tile([128, 512], F32, tag="dps")
                        nc.tensor.matmul(dps[:, 0:cw], lhsT=ones_sb[:],
                                         rhs=dt_sb[:, c0:c0 + cw],
                                         start=True, stop=True)
                        dps3 = _ap(dps[:], [dps[:].ap[0], [128, cw // 128],
                                            [1, 128]])
                        ioc3 = _ap(iotac_sb[:, 0:1],
                                   [iotac_sb[:].ap[0], [0, cw // 128], [0, 128]])
                        nc.vector.tensor_tensor(
                            out=ST_all[:, c0 // 128:(c0 + cw) // 128, :],
                            in0=dps3, in1=ioc3, op=ALU.is_equal)
                    zps = zpsp.tile([128, bpu, heads], F32)
                    for s in range(nbu):
                        Di = r_stream_D[r][u0 + s]
                        nc.tensor.matmul(zps[:, s, :], lhsT=ST_all[:, s, :],
                                         rhs=ad_all[:, Di, :],
                                         start=True, stop=True)

                # alpha = kew + a_src[src] + a_dst[dst]; lrelu; exp -> p
                zsb = zp.tile([128, bpu, heads], F32, tag="zsb")
                kew3 = _ap(kewv, [kewv.ap[0], [heads, bpu], [1, heads]])
                nc.vector.tensor_tensor(out=zsb[:], in0=kew3,
                                        in1=gt[:, :, HC:HC + heads], op=ALU.add)
                if admode == 'mm':
                    nc.vector.tensor_tensor(out=zsb[:, 0:nbu, :],
                                            in0=zsb[:, 0:nbu, :],
                                            in1=zps[:, 0:nbu, :], op=ALU.add)
                elif not cfg.get('no_gd'):
                    nc.vector.tensor_tensor(out=zsb[:], in0=zsb[:],
                                            in1=gd[:, :, doff:doff + heads],
                                            op=ALU.add)
                zl = zp.tile([128, bpu, heads], F32, tag="zl")
                nc.vector.tensor_scalar_mul(zl[:], zsb[:], NEG_SLOPE)
                nc.vector.tensor_tensor(out=zsb[:], in0=zsb[:], in1=zl[:],
                                        op=ALU.max)
                nc.scalar.activation(out=gt[:, :, HC:HC + heads], in_=zsb[:],
                                     func=mybir.ActivationFunctionType.Exp)

                # p-scale messages, then scatter-add per block
                g0 = gt[:, 0, 0:1]
                if cfg.get('ps4', True):
                    m4 = _ap(g0, [g0.ap[0], [rowe, bpu], [ch, heads], [1, ch]])
                    p4 = _ap(gt[:, 0, HC:HC + 1],
                             [g0.ap[0], [rowe, bpu], [1, heads], [0, ch]])
                    nc.vector.tensor_tensor(out=m4, in0=m4, in1=p4, op=ALU.mult)
                else:
                    for h in range(heads):
                        pb = gt[:, :, HC + h:HC + h + 1]
                        pb3 = _ap(pb, [pb.ap[0], pb.ap[1], [0, ch]])
                        nc.vector.tensor_tensor(out=gt[:, :, h * ch:(h + 1) * ch],
                                                in0=gt[:, :, h * ch:(h + 1) * ch],
                                                in1=pb3, op=ALU.mult)
                if no_mm:
                    continue
                for s in range(nbu):
                    Di = r_stream_D[r][u0 + s]
                    if Di not in ups_tiles:
                        ups_tiles[Di] = upsp.tile([128, HC + heads], F32,
                                                  name=f'ups{Di}', tag='ups')
                    first = (mm2_done[Di] == 0)
                    last = (mm2_done[Di] + 1 == mm2_total[Di])
                    nc.tensor.matmul(ups_tiles[Di][:], lhsT=S_all[:, s, :],
                                     rhs=gt[:, s, 0:HC + heads],
                                     start=first, stop=last,
                                     skip_group_check=True)
                    mm2_done[Di] += 1
                    if last:
                        finalize(Di, ups_tiles[Di])
                        del ups_tiles[Di]
            if no_mm:
                t = finp.tile([128, out_cols], OUTDT)
                nc.vector.memset(t[:], 0)
                for Di in range(n_dblk):
                    nc.sync.dma_start(out=outd[Di * 128:(Di + 1) * 128, :],
                                      in_=t[:])
    nc.compile()
    return nc


# ---------------------------------------------------------------- host plan

def make_plan(src, dst, ew, n_cores, shard, npad, n_ranges, bpu, k1, k2):
    range_size = npad // n_ranges
    n_dblk = shard // 128
    counts = np.zeros((n_cores, n_dblk, n_ranges), dtype=np.int64)
    core_of = dst // shard
    perm_pos = np.empty((n_cores, npad), np.int64)  # global node -> permuted pos
    for c in range(n_cores):
        base = c * shard
        pos = np.empty(npad, np.int64)
        pos[base:base + shard] = np.arange(shard)
        pos[:base] = shard + np.arange(base)
        pos[base + shard:] = np.arange(base + shard, npad)
        perm_pos[c] = pos

    edata = []
    for c in range(n_cores):
        m = core_of == c
        s_c, d_c, w_c = src[m], dst[m], ew[m]
        p_c = perm_pos[c][s_c]          # permuted src position
        base = c * shard
        drel = d_c - base
        D = drel // 128
        R = p_c // range_size
        edata.append((p_c, drel, w_c, D, R))
        for Di in range(n_dblk):
            mD = D == Di
            for r in range(n_ranges):
                counts[c, Di, r] = np.sum(mD & (R == r))
    nb = np.ceil(counts.max(axis=0) / 128).astype(np.int64)  # [n_dblk, n_ranges]
    for Di in range(n_dblk):
        if nb[Di].sum() == 0:
            nb[Di, 0] = 1

    r_stream_len = [int(nb[:, r].sum()) for r in range(n_ranges)]
    units = []
    for r in range(n_ranges):
        for u0 in range(0, r_stream_len[r], bpu):
            units.append((r, u0, min(bpu, r_stream_len[r] - u0)))
    n_blocks_total = sum(r_stream_len)
    ulen = bpu * 128
    wq = ulen // 16

    nb_cum = np.zeros((n_dblk + 1, n_ranges), np.int64)
    nb_cum[1:] = np.cumsum(nb, axis=0)
    ustarts = []
    rbase = 0
    for r in range(n_ranges):
        ustarts.append(rbase)
        rbase += r_stream_len[r]

    heads1 = len(k1)
    heads2 = len(k2)
    BB1 = 2 * wq + bpu + bpu * heads1
    BB2 = 2 * wq + bpu + bpu * heads2

    per_core = []
    for c in range(n_cores):
        p_c, drel_all, w_c, D, R = edata[c]
        nslots = n_blocks_total * 128
        idxS_all = np.zeros(nslots, np.int16)
        idxD_all = np.zeros(nslots, np.int16)
        dr_all = np.full(nslots, -1.0, np.float16)
        ew_all = np.zeros(nslots, np.float32)
        rbase = 0
        for r in range(n_ranges):
            for Di in range(n_dblk):
                sel = (D == Di) & (R == r)
                k = int(sel.sum())
                if k:
                    ss = p_c[sel] - r * range_size
                    o = np.argsort(ss, kind='stable')
                    start = (rbase + nb_cum[Di, r]) * 128
                    idxS_all[start:start + k] = ss[o].astype(np.int16)
                    dsel = drel_all[sel][o]
                    idxD_all[start:start + k] = dsel.astype(np.int16)
                    dr_all[start:start + k] = (dsel - Di * 128).astype(np.float16)
                    ew_all[start:start + k] = w_c[sel][o]
            rbase += r_stream_len[r]

        idxS_b = idxS_all.reshape(-1, 128)
        idxD_b = idxD_all.reshape(-1, 128)
        dr_b = dr_all.reshape(-1, 128)
        ew_b = ew_all.reshape(-1, 128)

        def wrap16(vals):
            """[ulen] int16 -> [128, ulen//16] wrapped + replicated."""
            w = np.zeros((16, len(vals) // 16), np.int16)
            ii = np.arange(len(vals))
            w[ii % 16, ii // 16] = vals
            return np.tile(w, (8, 1))

        blob1 = np.zeros((len(units), 128, BB1), np.int16)
        blob2 = np.zeros((len(units), 128, BB2), np.int16)
        dt = np.full((len(units), 1, ulen), -1.0, np.float16)
        for ui, (r, u0, nbu) in enumerate(units):
            b0 = ustarts[r] + u0
            iS = np.zeros(ulen, np.int16)
            iD = np.zeros(ulen, np.int16)
            iS[:nbu * 128] = idxS_b[b0:b0 + nbu].ravel()
            iD[:nbu * 128] = idxD_b[b0:b0 + nbu].ravel()
            for bl in (blob1, blob2):
                bl[ui, :, 0:wq] = wrap16(iS)
                bl[ui, :, wq:2 * wq] = wrap16(iD)
            dr_u = np.full((128, bpu), -1.0, np.float16)
            dr_u[:, :nbu] = dr_b[b0:b0 + nbu].T
            dt[ui, 0, :nbu * 128] = dr_b[b0:b0 + nbu].ravel()
            ew_u = np.zeros((128, bpu), np.float32)
            ew_u[:, :nbu] = ew_b[b0:b0 + nbu].T
            for bl, kk, hh in ((blob1, k1, heads1), (blob2, k2, heads2)):
                bl[ui, :, 2 * wq:2 * wq + bpu] = dr_u.view(np.int16)
                kew = (ew_u[:, :, None] * kk[None, None, :]).astype(np.float16)
                bl[ui, :, 2 * wq + bpu:2 * wq + bpu + bpu * hh] = \
                    kew.reshape(128, bpu * hh).view(np.int16)
        per_core.append(dict(blob1=blob1, blob2=blob2, dt=dt))

    return dict(nb=nb, units=units, n_blocks_total=n_blocks_total,
                bpu=bpu, per_core=per_core, perm_pos=perm_pos,
                n_ranges=n_ranges, range_size=range_size)


def blockdiag(att):
    """att [H, C] -> [H*C, H]"""
    H, C = att.shape
    A = np.zeros((H * C, H), np.float32)
    for h in range(H):
        A[h * C:(h + 1) * C, h] = att[h]
    return A


def gat_prepare(x, edge_index, edge_weight,
                W1, att_src1, att_dst1, att_edge1, We1, b1,
                W2, att_src2, att_dst2, att_edge2, We2, b2,
                n_cores=8, bpu=8):
    N, DIN = x.shape
    H, C1 = att_src1.shape
    C2 = att_src2.shape[1]
    shard = int(np.ceil(N / (128 * n_cores))) * 128
    npad = shard * n_cores
    n_ranges = 1
    while npad // n_ranges > 32767 or npad % n_ranges or (npad // n_ranges) % 128:
        n_ranges += 1
    range_size = npad // n_ranges

    src = np.asarray(edge_index[0], np.int64)
    dst = np.asarray(edge_index[1], np.int64)
    ew = np.asarray(edge_weight, np.float32)
    si = np.arange(N, dtype=np.int64)
    src2 = np.concatenate([src, si])
    dst2 = np.concatenate([dst, si])
    ew2 = np.concatenate([ew, np.full(N, ew.mean(), np.float32)])

    HC1 = H * C1
    k1 = np.array([np.dot(We1[0, h * C1:(h + 1) * C1], att_edge1[h])
                   for h in range(H)], np.float32)
    k2 = np.array([np.dot(We2[0], att_edge2[0])], np.float32)

    plan = make_plan(src2, dst2, ew2, n_cores, shard, npad, n_ranges, bpu,
                     k1, k2)

    rowe1 = 256 if HC1 + 2 * H > 128 else 128
    rowe2 = 256 if C2 + 2 > 128 else 128
    wcat1 = np.concatenate([W1, W1 @ blockdiag(att_src1), W1 @ blockdiag(att_dst1)],
                           axis=1).astype(np.float16)
    wcat2 = np.concatenate([W2, W2 @ att_src2.T, W2 @ att_dst2.T],
                           axis=1).astype(np.float16)
    iota = np.tile(np.arange(128, dtype=np.float16), (128, 1))
    iotac = np.zeros((128, 8), np.float16)
    iotac[:, 0] = np.arange(128, dtype=np.float16)

    common = dict(shard=shard, npad=npad, n_ranges=n_ranges,
                  range_size=range_size, nb=plan['nb'], units=plan['units'],
                  n_blocks_total=plan['n_blocks_total'], bpu=bpu, nq=1,
                  scratch=16384)
    cfg1 = dict(common, heads=H, ch=C1, rowe=rowe1, elu_out=True,
                out_cols=HC1, out_f16=True)
    cfg2 = dict(common, heads=1, ch=C2, rowe=rowe2, elu_out=False,
                out_cols=C2, out_f16=False)

    xpad = np.zeros((npad, DIN), np.float16)
    xpad[:N] = np.asarray(x, np.float16)

    meta = dict(N=N, npad=npad, shard=shard, n_cores=n_cores, plan=plan,
                H=H, C1=C1, C2=C2, HC1=HC1,
                wcat1=wcat1, wcat2=wcat2, k1=k1, k2=k2,
                b1=np.asarray(b1, np.float32), b2=np.asarray(b2, np.float32),
                iota=iota, iotac=iotac, cfg1=cfg1, cfg2=cfg2, xpad=xpad)
    return meta


def launch_inputs(meta, layer, dense_rows):
    """dense_rows: [npad, DIN] fp16 (x for L1, h for L2)."""
    plan = meta['plan']
    n_cores = meta['n_cores']
    HC = meta['HC1'] if layer == 1 else meta['C2']
    wcat = meta['wcat1'] if layer == 1 else meta['wcat2']
    b = meta['b1'] if layer == 1 else meta['b2']
    in_maps = []
    for c in range(n_cores):
        perm = np.argsort(plan['perm_pos'][c], kind='stable')  # pos -> node
        xp = dense_rows[perm]
        in_maps.append(dict(
            xT=np.ascontiguousarray(xp.T),
            wcat=wcat,
            brep=np.tile(b, (128, 1)).astype(np.float32),
            iota=meta['iota'],
            iotac=meta['iotac'],
            blob=plan['per_core'][c][f'blob{layer}'],
            dt=plan['per_core'][c]['dt'],
        ))
    return in_maps


class SpmdRunner:
    def __init__(self, nc, n_cores=8):
        install_neuronx_cc_hook()
        self.nc = nc
        self.n_cores = n_cores
        partition_name = nc.partition_id_tensor.name if nc.partition_id_tensor else None
        in_names, out_names, out_avals, zero_outs = [], [], [], []
        for alloc in nc.m.functions[0].allocations:
            if not isinstance(alloc, mybir.MemoryLocationSet):
                continue
            name = alloc.memorylocations[0].name
            if alloc.kind == "ExternalInput":
                if name != partition_name:
                    in_names.append(name)
            elif alloc.kind == "ExternalOutput":
                out_names.append(name)
                shape = tuple(alloc.tensor_shape)
                dtype = mybir.dt.np(alloc.dtype)
                out_avals.append(jax.core.ShapedArray(shape, dtype))
                zero_outs.append(np.zeros(shape, dtype))
        self.in_names = list(in_names)
        self.out_names = out_names
        self.out_avals = out_avals
        self.zero_outs = zero_outs
        n_params = len(in_names)
        n_outs = len(out_avals)
        all_in_names = in_names + out_names
        if partition_name is not None:
            all_in_names.append(partition_name)

        def _body(*args):
            operands = list(args)
            if partition_name is not None:
                operands.append(partition_id_tensor())
            outs = _bass_exec_p.bind(
                *operands,
                out_avals=tuple(out_avals),
                in_names=tuple(all_in_names),
                out_names=tuple(out_names),
                lowering_input_output_aliases=(),
                sim_require_finite=False,
                sim_require_nnan=False,
                nc=nc,
            )
            return tuple(outs)

        devices = jax.devices()[:n_cores]
        self.mesh = Mesh(np.asarray(devices), ("core",))
        in_specs = (PartitionSpec("core"),) * (n_params + n_outs)
        out_specs = (PartitionSpec("core"),) * n_outs
        self.fn = jax.jit(
            shard_map(_body, mesh=self.mesh, in_specs=in_specs,
                      out_specs=out_specs, check_rep=False),
            keep_unused=True,
        )
        self._dev_args = None

    def stage(self, in_maps):
        n = self.n_cores
        concat_in = [
            np.concatenate([np.asarray(in_maps[c][name]) for c in range(n)], axis=0)
            for name in self.in_names
        ]
        concat_zeros = [
            np.zeros((n * z.shape[0], *z.shape[1:]), z.dtype) for z in self.zero_outs
        ]
        self._dev_args = [jax.device_put(a) for a in concat_in + concat_zeros]

    def run(self):
        outs = self.fn(*self._dev_args)
        jax.block_until_ready(outs)
        return outs

    def results(self, outs):
        n = self.n_cores
        return [
            {name: np.asarray(outs[i]).reshape(n, *self.out_avals[i].shape)[c]
             for i, name in enumerate(self.out_names)}
            for c in range(n)
        ]

    def time_it(self, iters=5):
        self.run()
        ts = []
        for _ in range(iters):
            t0 = time.perf_counter()
            self.run()
            ts.append(time.perf_counter() - t0)
        return min(ts), ts


def build_floor(cfg, n_units):
    """Trivial kernel with identical I/O decls, for dispatch-floor timing."""
    heads = cfg['heads']
    HC = heads * cfg['ch']
    npad = cfg['npad']
    bpu = cfg['bpu']
    ulen = bpu * 128
    wq = ulen // 16
    BB = 2 * wq + bpu + bpu * heads
    OUTDT = F16 if cfg['out_f16'] else F32
    nc = bacc.Bacc(target_bir_lowering=False)
    xT = nc.dram_tensor("xT", [128, npad], F16, kind="ExternalInput")
    nc.dram_tensor("wcat", [128, HC + 2 * heads], F16, kind="ExternalInput")
    nc.dram_tensor("brep", [128, HC], F32, kind="ExternalInput")
    nc.dram_tensor("iota", [128, 128], F16, kind="ExternalInput")
    nc.dram_tensor("blob", [n_units, 128, BB], I16, kind="ExternalInput")
    outd = nc.dram_tensor("out", [cfg['shard'], cfg['out_cols']], OUTDT,
                          kind="ExternalOutput")
    with tile.TileContext(nc) as tc:
        with tc.tile_pool(name="s", bufs=2) as pool:
            t0 = pool.tile([128, 128], F16)
            nc.sync.dma_start(out=t0[:], in_=xT[:, 0:128])
            t = pool.tile([128, cfg['out_cols']], OUTDT)
            nc.vector.memset(t[:], 0)
            for ci in range(cfg['shard'] // 128):
                nc.sync.dma_start(out=outd[ci * 128:(ci + 1) * 128, :], in_=t[:])
    nc.compile()
    return nc


def kernel(**inputs):
    inputs = {k: np.asarray(v) for k, v in inputs.items()}
    x = inputs['x'].astype(np.float32)
    edge_index = inputs['edge_index'].astype(np.int64)
    ew = inputs['edge_weight'].astype(np.float32)
    meta = gat_prepare(
        x, edge_index, ew,
        inputs['W1'].astype(np.float32), inputs['att_src1'].astype(np.float32),
        inputs['att_dst1'].astype(np.float32), inputs['att_edge1'].astype(np.float32),
        inputs['We1'].astype(np.float32), inputs['b1'].astype(np.float32),
        inputs['W2'].astype(np.float32), inputs['att_src2'].astype(np.float32),
        inputs['att_dst2'].astype(np.float32), inputs['att_edge2'].astype(np.float32),
        inputs['We2'].astype(np.float32), inputs['b2'].astype(np.float32))
    nc1 = build_launch(meta['cfg1'])
    nc2 = build_launch(meta['cfg2'])
    N, shard, n_cores = meta['N'], meta['shard'], meta['n_cores']

    r1 = SpmdRunner(nc1, n_cores)
    r1.stage(launch_inputs(meta, 1, meta['xpad']))
    res1 = r1.results(r1.run())
    hfull = np.concatenate([res1[c]['out'] for c in range(n_cores)], axis=0)
    hfull[N:] = 0

    r2 = SpmdRunner(nc2, n_cores)
    r2.stage(launch_inputs(meta, 2, hfull))
    res2 = r2.results(r2.run())
    out = np.concatenate([res2[c]['out'] for c in range(n_cores)], axis=0)[:N]

    floor_r = None
    try:
        ncf = build_floor(meta['cfg1'], len(meta['plan']['units']))
        floor_r = SpmdRunner(ncf, n_cores)
        floor_r.stage(launch_inputs(meta, 1, meta['xpad']))
        floor_r.run()
    except Exception:
        floor_r = None
    kernel._last = dict(meta=meta, r1=r1, r2=r2, nc1=nc1, nc2=nc2, floor=floor_r)
    return out.astype(np.float32)


# revision 5
# speedup vs baseline: 2.1035x; 1.4451x over previous
"""Self-contained 2-layer GAT kernel for Trainium2 (8 NeuronCores), v2.

Destination-sharded (each core owns its 12544-dst shard; no collectives).
Per layer the dense node table xs_tab[n, [msg | a_src | a_dst | pad]] is
computed in fp16 (phase 1, batched DMAs), then edges — bucketed host-side by
(dst-block of 128, src-range) into 128-edge blocks, bpu blocks per unit —
are processed with TWO SWDGE gathers per unit (source rows for messages +
alpha_src; destination rows for alpha_dst), one packed metadata blob DMA
(int16 gather indices, dst-rel row as fp16, k·edge_weight as fp16), one
batched one-hot build (is_equal vs iota), a short batched alpha pipeline
(add, lrelu, exp), and one PE matmul per 128-edge block that scatter-adds
p-scaled messages and the softmax denominator into the dst-block PSUM
accumulator. Finalize normalizes by the accumulated denominator, adds bias
(+ ELU for layer 1) and writes the dst shard. fp16 data / f32 accumulate.
"""
import sys
sys.path.insert(0, '/opt/trn_rl_repo')
import time
import numpy as np
import jax
from jax.sharding import Mesh, PartitionSpec
from jax.experimental.shard_map import shard_map

import concourse.bass as bass
import concourse.tile as tile
from concourse import bacc, mybir
from concourse.library_config import mlp as mlp_lib
from concourse.bass2jax import install_neuronx_cc_hook, _bass_exec_p, partition_id_tensor

F32 = mybir.dt.float32
F16 = mybir.dt.float16
I16 = mybir.dt.int16
NEG_SLOPE = 0.2
EPS = 1e-16
ALU = mybir.AluOpType


def _ap(src, dims):
    """Build an AP over src's tensor with explicit [stride, size] dims."""
    return bass.AP(tensor=src.tensor, offset=src.offset,
                   ap=[list(d) for d in dims])


def build_launch(cfg):
    """cfg keys: shard, npad, n_ranges, range_size, heads, ch, rowe,
    nb, units, n_blocks_total, bpu, elu_out, out_cols, out_f16, nq, scratch"""
    heads, ch = cfg['heads'], cfg['ch']
    HC = heads * ch
    shard, npad = cfg['shard'], cfg['npad']
    n_dblk = shard // 128
    n_chunks = npad // 128
    rowe = cfg['rowe']                  # fp16 elems per table row
    wcols = HC + 2 * heads
    assert wcols <= rowe
    dbase = 128 if HC + heads >= 128 else 0   # col window for the dst gather
    doff = HC + heads - dbase                 # a_dst col inside gd tile
    assert doff + heads <= 128
    nb = cfg['nb']
    units = cfg['units']
    bpu = cfg['bpu']
    ulen = bpu * 128
    wq = ulen // 16
    n_units = len(units)
    n_ranges, range_size = cfg['n_ranges'], cfg['range_size']
    out_cols = cfg['out_cols']
    OUTDT = F16 if cfg['out_f16'] else F32
    nq = cfg.get('nq', 4)
    BB = 2 * wq + bpu + bpu * heads     # blob int16 cols

    admode = cfg.get('admode', 'mm')
    nc = bacc.Bacc(target_bir_lowering=False, num_swdge_queues=nq,
                   dynamic_dma_scratch_size=cfg.get('scratch', 16384),
                   detect_race_conditions=not cfg.get('prep'))
    xT = nc.dram_tensor("xT", [128, npad], F16, kind="ExternalInput")
    wcat = nc.dram_tensor("wcat", [128, wcols], F16, kind="ExternalInput")
    brepd = nc.dram_tensor("brep", [128, HC], F32, kind="ExternalInput")
    iotad = nc.dram_tensor("iota", [128, 128], F16, kind="ExternalInput")
    blobd = nc.dram_tensor("blob", [n_units, 128, BB], I16, kind="ExternalInput")
    if admode == 'mm':
        dtd = nc.dram_tensor("dt", [n_units, 1, ulen], F16, kind="ExternalInput")
        iotacd = nc.dram_tensor("iotac", [128, 8], F16, kind="ExternalInput")
    outd = nc.dram_tensor("out", [shard, out_cols], OUTDT, kind="ExternalOutput")

    xs_tab = nc.dram_tensor("xs_tab", [npad, rowe], F16)

    nc.gpsimd.load_library(mlp_lib)

    with tile.TileContext(nc) as tc:
        # ---------------- phase 1: dense node table (replicated over cores)
        CB = 16
        assert n_chunks % CB == 0, (n_chunks, CB)
        with (
            tc.tile_pool(name="p1c", bufs=1) as p1c,
            tc.tile_pool(name="p1x", bufs=3) as p1x,
            tc.tile_pool(name="p1s", bufs=3) as p1s,
            tc.tile_pool(name="p1ps", bufs=8, space="PSUM") as p1ps,
        ):
            wc_sb = p1c.tile([128, wcols], F16)
            nc.sync.dma_start(out=wc_sb[:], in_=wcat[:])
            for it in range(n_chunks // CB):
                xt = p1x.tile([128, CB, 128], F16)
                nc.sync.dma_start(
                    out=xt[:], in_=xT[:, it * CB * 128:(it + 1) * CB * 128])
                st = p1s.tile([128, CB, rowe], F16)
                if rowe > wcols:
                    nc.vector.memset(st[:, :, wcols:rowe], 0)
                for j in range(CB):
                    ps = p1ps.tile([128, wcols], F32)
                    nc.tensor.matmul(ps[:], lhsT=xt[:, j, :], rhs=wc_sb[:],
                                     start=True, stop=True)
                    nc.vector.tensor_copy(st[:, j, 0:wcols], ps[:])
                # rows it*CB*128 + j*128 + p  <-  st[p, j, :]
                dview = _ap(xs_tab[it * CB * 128:(it + 1) * CB * 128, :],
                            [[rowe, 128], [128 * rowe, CB], [1, rowe]])
                nc.sync.dma_start(out=dview, in_=st[:])

        tc.strict_bb_all_engine_barrier()

        # max concurrently-open dst-block accumulators under the chosen order
        _r_stream_D = []
        for _r in range(n_ranges):
            _l = []
            for _Di in range(n_dblk):
                _l += [_Di] * int(nb[_Di, _r])
            _r_stream_D.append(_l)
        _order = sorted(range(n_units),
                        key=lambda ui: (_r_stream_D[units[ui][0]][units[ui][1]],
                                        units[ui][0]))
        _tot = {d: int(nb[d].sum()) for d in range(n_dblk)}
        _done = {d: 0 for d in range(n_dblk)}
        _open, _max_open = set(), 1
        for _ui in _order:
            _r, _u0, _nbu = units[_ui]
            for _s in range(_nbu):
                _d = _r_stream_D[_r][_u0 + _s]
                _open.add(_d)
                _done[_d] += 1
                if _done[_d] == _tot[_d]:
                    _open.discard(_d)
                _max_open = max(_max_open, len(_open))
        ups_bufs = max(2, _max_open)
        assert ups_bufs + 3 <= 8, (ups_bufs, "PSUM banks over budget")

        # ---------------- phase 2: edge pipeline
        with (
            tc.tile_pool(name="cst", bufs=1) as cst,
            tc.tile_pool(name="blp", bufs=4) as blp,
            tc.tile_pool(name="gx", bufs=3) as gx,
            tc.tile_pool(name="gd", bufs=3) as gdp,
            tc.tile_pool(name="sal", bufs=3) as salp,
            tc.tile_pool(name="dtp", bufs=3) as dtp,
            tc.tile_pool(name="stp", bufs=3) as stp,
            tc.tile_pool(name="zp", bufs=3) as zp,
            tc.tile_pool(name="fin", bufs=2) as finp,
            tc.tile_pool(name="ups", bufs=ups_bufs, space="PSUM") as upsp,
            tc.tile_pool(name="dps", bufs=2, space="PSUM") as dpsp,
            tc.tile_pool(name="zps", bufs=1, space="PSUM") as zpsp,
        ):
            iota_sb = cst.tile([128, 128], F16)
            nc.sync.dma_start(out=iota_sb[:], in_=iotad[:])
            brep_sb = cst.tile([128, HC], F32)
            nc.sync.dma_start(out=brep_sb[:], in_=brepd[:])
            if admode == 'mm':
                iotac_sb = cst.tile([128, 8], F16)
                nc.sync.dma_start(out=iotac_sb[:], in_=iotacd[:])
                ones_sb = cst.tile([1, 128], F16)
                nc.vector.memset(ones_sb[:], 1.0)
                # a_dst for the core's own dst shard, [q, Di, h] from xs_tab
                ad_all = cst.tile([128, n_dblk, heads], F16)
                adview = _ap(xs_tab[0:shard, HC + heads:HC + 2 * heads],
                             [[rowe, 128], [128 * rowe, n_dblk], [1, heads]])
                nc.sync.dma_start(out=ad_all[:], in_=adview)

            mm2_total = {Di: int(nb[Di].sum()) for Di in range(n_dblk)}
            mm2_done = {Di: 0 for Di in range(n_dblk)}
            ups_tiles = {}

            # D of each position in each r stream (r-streams are D-major)
            r_stream_D = []
            for r in range(n_ranges):
                lst = []
                for Di in range(n_dblk):
                    lst += [Di] * int(nb[Di, r])
                r_stream_D.append(lst)

            order = sorted(range(n_units),
                           key=lambda ui: (r_stream_D[units[ui][0]][units[ui][1]],
                                           units[ui][0]))

            def finalize(Di, ups):
                sr = finp.tile([128, heads], F32, tag="sr")
                nc.vector.tensor_scalar_add(sr[:], ups[:, HC:HC + heads], EPS)
                rr = finp.tile([128, heads], F32, tag="rr")
                nc.vector.reciprocal(rr[:], sr[:])
                h0 = finp.tile([128, HC], F32, tag="h0")
                rb = _ap(rr[:], [rr[:].ap[0], rr[:].ap[1], [0, ch]])
                nc.vector.tensor_tensor(
                    out=h0[:].rearrange("p (h c) -> p h c", h=heads),
                    in0=ups[:, 0:HC].rearrange("p (h c) -> p h c", h=heads),
                    in1=rb, op=ALU.mult)
                nc.vector.tensor_tensor(out=h0[:], in0=h0[:], in1=brep_sb[:],
                                        op=ALU.add)
                res = finp.tile([128, out_cols], OUTDT, tag="res")
                if cfg['elu_out']:
                    m0 = finp.tile([128, HC], F32, tag="m0")
                    nc.vector.tensor_scalar_min(m0[:], h0[:], 0.0)
                    e = finp.tile([128, HC], F32, tag="e")
                    nc.scalar.activation(out=e[:], in_=m0[:],
                                         func=mybir.ActivationFunctionType.Exp)
                    nc.vector.tensor_scalar_add(e[:], e[:], -1.0)
                    nc.vector.tensor_scalar_max(h0[:], h0[:], 0.0)
                    nc.vector.tensor_tensor(out=res[:], in0=e[:], in1=h0[:],
                                            op=ALU.add)
                else:
                    nc.vector.tensor_copy(res[:], h0[:, 0:out_cols])
                nc.sync.dma_start(out=outd[Di * 128:(Di + 1) * 128, :],
                                  in_=res[:])

            no_vec = cfg.get('no_vec')
            no_mm = cfg.get('no_mm') or no_vec
            prep = cfg.get('prep')
            pd = cfg.get('pd', 1)
            dma_sem = nc.alloc_semaphore("gsem") if prep else None
            q2 = cfg.get('q2') and nq >= 2
            prev_gather = None
            gidx = 0
            if cfg.get('p1_only'):
                order = []
                no_mm = True
            for k, ui in enumerate(order):
                r, u0, nbu = units[ui]
                blob = blp.tile([128, BB], I16)
                nc.sync.dma_start(out=blob[:], in_=blobd[ui])
                idxS = blob[:, 0:wq]
                idxD = blob[:, wq:2 * wq]
                drv = blob[:, 2 * wq:2 * wq + bpu].bitcast(F16)
                kewv = blob[:, 2 * wq + bpu:2 * wq + bpu + bpu * heads].bitcast(F16)

                # Fixed queue per gather type: Pool-engine DMAs are emitted
                # strictly as (S, D) pairs, so Tile's 8-slot DMASW rotation
                # puts all S gathers on even slots and all D gathers on odd
                # slots — a constant queue per type keeps every DMASW sem
                # locked to a single SWDGE queue.
                gt = gx.tile([128, bpu, rowe], F16)
                gd = gdp.tile([128, bpu, 128], F16) if admode == 'gather' else None
                if admode == 'mm':
                    dt_sb = dtp.tile([1, ulen], F16)
                    nc.sync.dma_start(out=dt_sb[:], in_=dtd[ui])
                if prep:
                    # prepare_only pipelining: desc-gen of unit k overlaps the
                    # in-flight transfers of units k-pd+1..k-1; the Pool-side
                    # wait bounds the SWDGE ring to pd units' descriptors.
                    if k >= pd:
                        nc.gpsimd.wait_ge(dma_sem, 32 * (k - pd + 1))
                    nc.gpsimd.dma_gather(
                        out_ap=gt[:],
                        in_ap=xs_tab[r * range_size:(r + 1) * range_size, :],
                        idxs_ap=idxS, num_idxs=ulen, num_idxs_reg=ulen,
                        elem_size=rowe, queue_num=0,
                        prepare_only=True, sem=dma_sem)
                    nc.gpsimd.dma_gather(
                        out_ap=gd[:],
                        in_ap=xs_tab[0:shard, dbase:dbase + 128],
                        idxs_ap=idxD, num_idxs=ulen, num_idxs_reg=ulen,
                        elem_size=128, elem_step=rowe, queue_num=0,
                        prepare_only=True, sem=dma_sem)
                    nc.gpsimd.trigger_dma(count=None)
                    if not no_vec:
                        nc.vector.wait_ge(dma_sem, 32 * (k + 1))
                else:
                    # q2: alternate queues in BUILD order and chain each
                    # gather to the previous one with a no-sync dep so the
                    # scheduler preserves their relative order — keeps every
                    # DMASW sem slot (assigned round-robin in scheduled
                    # order) locked to a single SWDGE queue.
                    gi = nc.gpsimd.dma_gather(
                        out_ap=gt[:],
                        in_ap=xs_tab[r * range_size:(r + 1) * range_size, :],
                        idxs_ap=idxS, num_idxs=ulen, num_idxs_reg=ulen,
                        elem_size=rowe, queue_num=(gidx % 2) if q2 else 0)
                    gidx += 1
                    if q2:
                        if prev_gather is not None:
                            s = bass.InstructionNameOrderedSet()
                            s.add(prev_gather.ins.name)
                            gi.ins.add_nosync_dependencies_from(s)
                        prev_gather = gi
                    if admode == 'gather' and not cfg.get('no_gd'):
                        nc.gpsimd.dma_gather(
                            out_ap=gd[:],
                            in_ap=xs_tab[0:shard, dbase:dbase + 128],
                            idxs_ap=idxD, num_idxs=ulen, num_idxs_reg=ulen,
                            elem_size=128, elem_step=rowe, queue_num=0)

                if no_vec:
                    continue
                # one-hot S for all bpu blocks: S[e, s, d] = (dr[e,s] == d)
                S_all = salp.tile([128, bpu, 128], F16)
                dr3 = _ap(drv, [drv.ap[0], [1, bpu], [0, 128]])
                io3 = _ap(iota_sb[:], [iota_sb[:].ap[0], [0, bpu], [1, 128]])
                nc.vector.tensor_tensor(out=S_all[:], in0=dr3, in1=io3,
                                        op=ALU.is_equal)

                if admode == 'mm':
                    # S^T one-hot [d, s, q] built from the partition-broadcast
                    # of dr (PE outer product with a ones column), then
                    # alpha_dst per edge via one tiny matmul per block.
                    # 512-col chunks: a matmul output must stay in one PSUM bank.
                    ST_all = stp.tile([128, bpu, 128], F16)
                    for c0 in range(0, ulen, 512):
                        cw = min(512, ulen - c0)
                        dps = dpsp.tile([128, 512], F32, tag="dps")
                        nc.tensor.matmul(dps[:, 0:cw], lhsT=ones_sb[:],
                                         rhs=dt_sb[:, c0:c0 + cw],
                                         start=True, stop=True)
                        dps3 = _ap(dps[:], [dps[:].ap[0], [128, cw // 128],
                                            [1, 128]])
                        ioc3 = _ap(iotac_sb[:, 0:1],
                                   [iotac_sb[:].ap[0], [0, cw // 128], [0, 128]])
                        nc.vector.tensor_tensor(
                            out=ST_all[:, c0 // 128:(c0 + cw) // 128, :],
                            in0=dps3, in1=ioc3, op=ALU.is_equal)
                    zps = zpsp.tile([128, bpu, heads], F32)
                    for s in range(nbu):
                        Di = r_stream_D[r][u0 + s]
                        nc.tensor.matmul(zps[:, s, :], lhsT=ST_all[:, s, :],
                                         rhs=ad_all[:, Di, :],
                                         start=True, stop=True)

                # alpha = kew + a_src[src] + a_dst[dst]; lrelu; exp -> p
                zsb = zp.tile([128, bpu, heads], F32, tag="zsb")
                kew3 = _ap(kewv, [kewv.ap[0], [heads, bpu], [1, heads]])
                nc.vector.tensor_tensor(out=zsb[:], in0=kew3,
                                        in1=gt[:, :, HC:HC + heads], op=ALU.add)
                if admode == 'mm':
                    nc.vector.tensor_tensor(out=zsb[:, 0:nbu, :],
                                            in0=zsb[:, 0:nbu, :],
                                            in1=zps[:, 0:nbu, :], op=ALU.add)
                elif not cfg.get('no_gd'):
                    nc.vector.tensor_tensor(out=zsb[:], in0=zsb[:],
                                            in1=gd[:, :, doff:doff + heads],
                                            op=ALU.add)
                zl = zp.tile([128, bpu, heads], F32, tag="zl")
                nc.vector.tensor_scalar_mul(zl[:], zsb[:], NEG_SLOPE)
                nc.vector.tensor_tensor(out=zsb[:], in0=zsb[:], in1=zl[:],
                                        op=ALU.max)
                nc.scalar.activation(out=gt[:, :, HC:HC + heads], in_=zsb[:],
                                     func=mybir.ActivationFunctionType.Exp)

                # p-scale messages, then scatter-add per block
                g0 = gt[:, 0, 0:1]
                if cfg.get('ps4', True):
                    m4 = _ap(g0, [g0.ap[0], [rowe, bpu], [ch, heads], [1, ch]])
                    p4 = _ap(gt[:, 0, HC:HC + 1],
                             [g0.ap[0], [rowe, bpu], [1, heads], [0, ch]])
                    nc.vector.tensor_tensor(out=m4, in0=m4, in1=p4, op=ALU.mult)
                else:
                    for h in range(heads):
                        pb = gt[:, :, HC + h:HC + h + 1]
                        pb3 = _ap(pb, [pb.ap[0], pb.ap[1], [0, ch]])
                        nc.vector.tensor_tensor(out=gt[:, :, h * ch:(h + 1) * ch],
                                                in0=gt[:, :, h * ch:(h + 1) * ch],
                                                in1=pb3, op=ALU.mult)
                if no_mm:
                    continue
                for s in range(nbu):
                    Di = r_stream_D[r][u0 + s]
                    if Di not in ups_tiles:
                        ups_tiles[Di] = upsp.tile([128, HC + heads], F32,
                                                  name=f'ups{Di}', tag='ups')
                    first = (mm2_done[Di] == 0)
                    last = (mm2_done[Di] + 1 == mm2_total[Di])
                    nc.tensor.matmul(ups_tiles[Di][:], lhsT=S_all[:, s, :],
                                     rhs=gt[:, s, 0:HC + heads],
                                     start=first, stop=last,
                                     skip_group_check=True)
                    mm2_done[Di] += 1
                    if last:
                        finalize(Di, ups_tiles[Di])
                        del ups_tiles[Di]
            if no_mm:
                t = finp.tile([128, out_cols], OUTDT)
                nc.vector.memset(t[:], 0)
                for Di in range(n_dblk):
                    nc.sync.dma_start(out=outd[Di * 128:(Di + 1) * 128, :],
                                      in_=t[:])
    nc.compile()
    return nc


# ---------------------------------------------------------------- host plan

def make_plan(src, dst, ew, n_cores, shard, npad, n_ranges, bpu, k1, k2):
    range_size = npad // n_ranges
    n_dblk = shard // 128
    counts = np.zeros((n_cores, n_dblk, n_ranges), dtype=np.int64)
    core_of = dst // shard
    perm_pos = np.empty((n_cores, npad), np.int64)  # global node -> permuted pos
    for c in range(n_cores):
        base = c * shard
        pos = np.empty(npad, np.int64)
        pos[base:base + shard] = np.arange(shard)
        pos[:base] = shard + np.arange(base)
        pos[base + shard:] = np.arange(base + shard, npad)
        perm_pos[c] = pos

    edata = []
    for c in range(n_cores):
        m = core_of == c
        s_c, d_c, w_c = src[m], dst[m], ew[m]
        p_c = perm_pos[c][s_c]          # permuted src position
        base = c * shard
        drel = d_c - base
        D = drel // 128
        R = p_c // range_size
        edata.append((p_c, drel, w_c, D, R))
        for Di in range(n_dblk):
            mD = D == Di
            for r in range(n_ranges):
                counts[c, Di, r] = np.sum(mD & (R == r))
    nb = np.ceil(counts.max(axis=0) / 128).astype(np.int64)  # [n_dblk, n_ranges]
    for Di in range(n_dblk):
        if nb[Di].sum() == 0:
            nb[Di, 0] = 1

    r_stream_len = [int(nb[:, r].sum()) for r in range(n_ranges)]
    units = []
    for r in range(n_ranges):
        for u0 in range(0, r_stream_len[r], bpu):
            units.append((r, u0, min(bpu, r_stream_len[r] - u0)))
    n_blocks_total = sum(r_stream_len)
    ulen = bpu * 128
    wq = ulen // 16

    nb_cum = np.zeros((n_dblk + 1, n_ranges), np.int64)
    nb_cum[1:] = np.cumsum(nb, axis=0)
    ustarts = []
    rbase = 0
    for r in range(n_ranges):
        ustarts.append(rbase)
        rbase += r_stream_len[r]

    heads1 = len(k1)
    heads2 = len(k2)
    BB1 = 2 * wq + bpu + bpu * heads1
    BB2 = 2 * wq + bpu + bpu * heads2

    per_core = []
    for c in range(n_cores):
        p_c, drel_all, w_c, D, R = edata[c]
        nslots = n_blocks_total * 128
        idxS_all = np.zeros(nslots, np.int16)
        idxD_all = np.zeros(nslots, np.int16)
        dr_all = np.full(nslots, -1.0, np.float16)
        ew_all = np.zeros(nslots, np.float32)
        rbase = 0
        for r in range(n_ranges):
            for Di in range(n_dblk):
                sel = (D == Di) & (R == r)
                k = int(sel.sum())
                if k:
                    ss = p_c[sel] - r * range_size
                    o = np.argsort(ss, kind='stable')
                    start = (rbase + nb_cum[Di, r]) * 128
                    idxS_all[start:start + k] = ss[o].astype(np.int16)
                    dsel = drel_all[sel][o]
                    idxD_all[start:start + k] = dsel.astype(np.int16)
                    dr_all[start:start + k] = (dsel - Di * 128).astype(np.float16)
                    ew_all[start:start + k] = w_c[sel][o]
            rbase += r_stream_len[r]

        idxS_b = idxS_all.reshape(-1, 128)
        idxD_b = idxD_all.reshape(-1, 128)
        dr_b = dr_all.reshape(-1, 128)
        ew_b = ew_all.reshape(-1, 128)

        def wrap16(vals):
            """[ulen] int16 -> [128, ulen//16] wrapped + replicated."""
            w = np.zeros((16, len(vals) // 16), np.int16)
            ii = np.arange(len(vals))
            w[ii % 16, ii // 16] = vals
            return np.tile(w, (8, 1))

        blob1 = np.zeros((len(units), 128, BB1), np.int16)
        blob2 = np.zeros((len(units), 128, BB2), np.int16)
        dt = np.full((len(units), 1, ulen), -1.0, np.float16)
        for ui, (r, u0, nbu) in enumerate(units):
            b0 = ustarts[r] + u0
            iS = np.zeros(ulen, np.int16)
            iD = np.zeros(ulen, np.int16)
            iS[:nbu * 128] = idxS_b[b0:b0 + nbu].ravel()
            iD[:nbu * 128] = idxD_b[b0:b0 + nbu].ravel()
            for bl in (blob1, blob2):
                bl[ui, :, 0:wq] = wrap16(iS)
                bl[ui, :, wq:2 * wq] = wrap16(iD)
            dr_u = np.full((128, bpu), -1.0, np.float16)
            dr_u[:, :nbu] = dr_b[b0:b0 + nbu].T
            dt[ui, 0, :nbu * 128] = dr_b[b0:b0 + nbu].ravel()
            ew_u = np.zeros((128, bpu), np.float32)
            ew_u[:, :nbu] = ew_b[b0:b0 + nbu].T
            for bl, kk, hh in ((blob1, k1, heads1), (blob2, k2, heads2)):
                bl[ui, :, 2 * wq:2 * wq + bpu] = dr_u.view(np.int16)
                kew = (ew_u[:, :, None] * kk[None, None, :]).astype(np.float16)
                bl[ui, :, 2 * wq + bpu:2 * wq + bpu + bpu * hh] = \
                    kew.reshape(128, bpu * hh).view(np.int16)
        per_core.append(dict(blob1=blob1, blob2=blob2, dt=dt))

    return dict(nb=nb, units=units, n_blocks_total=n_blocks_total,
                bpu=bpu, per_core=per_core, perm_pos=perm_pos,
                n_ranges=n_ranges, range_size=range_size)


def blockdiag(att):
    """att [H, C] -> [H*C, H]"""
    H, C = att.shape
    A = np.zeros((H * C, H), np.float32)
    for h in range(H):
        A[h * C:(h + 1) * C, h] = att[h]
    return A


def gat_prepare(x, edge_index, edge_weight,
                W1, att_src1, att_dst1, att_edge1, We1, b1,
                W2, att_src2, att_dst2, att_edge2, We2, b2,
                n_cores=8, bpu=8):
    N, DIN = x.shape
    H, C1 = att_src1.shape
    C2 = att_src2.shape[1]
    shard = int(np.ceil(N / (128 * n_cores))) * 128
    npad = shard * n_cores
    n_ranges = 1
    while npad // n_ranges > 32767 or npad % n_ranges or (npad // n_ranges) % 128:
        n_ranges += 1
    range_size = npad // n_ranges

    src = np.asarray(edge_index[0], np.int64)
    dst = np.asarray(edge_index[1], np.int64)
    ew = np.asarray(edge_weight, np.float32)
    si = np.arange(N, dtype=np.int64)
    src2 = np.concatenate([src, si])
    dst2 = np.concatenate([dst, si])
    ew2 = np.concatenate([ew, np.full(N, ew.mean(), np.float32)])

    HC1 = H * C1
    k1 = np.array([np.dot(We1[0, h * C1:(h + 1) * C1], att_edge1[h])
                   for h in range(H)], np.float32)
    k2 = np.array([np.dot(We2[0], att_edge2[0])], np.float32)

    plan = make_plan(src2, dst2, ew2, n_cores, shard, npad, n_ranges, bpu,
                     k1, k2)

    rowe1 = 256 if HC1 + 2 * H > 128 else 128
    rowe2 = 256 if C2 + 2 > 128 else 128
    wcat1 = np.concatenate([W1, W1 @ blockdiag(att_src1), W1 @ blockdiag(att_dst1)],
                           axis=1).astype(np.float16)
    wcat2 = np.concatenate([W2, W2 @ att_src2.T, W2 @ att_dst2.T],
                           axis=1).astype(np.float16)
    iota = np.tile(np.arange(128, dtype=np.float16), (128, 1))
    iotac = np.zeros((128, 8), np.float16)
    iotac[:, 0] = np.arange(128, dtype=np.float16)

    common = dict(shard=shard, npad=npad, n_ranges=n_ranges,
                  range_size=range_size, nb=plan['nb'], units=plan['units'],
                  n_blocks_total=plan['n_blocks_total'], bpu=bpu, nq=1,
                  scratch=16384)
    cfg1 = dict(common, heads=H, ch=C1, rowe=rowe1, elu_out=True,
                out_cols=HC1, out_f16=True)
    cfg2 = dict(common, heads=1, ch=C2, rowe=rowe2, elu_out=False,
                out_cols=C2, out_f16=False)

    xpad = np.zeros((npad, DIN), np.float16)
    xpad[:N] = np.asarray(x, np.float16)

    meta = dict(N=N, npad=npad, shard=shard, n_cores=n_cores, plan=plan,
                H=H, C1=C1, C2=C2, HC1=HC1,
                wcat1=wcat1, wcat2=wcat2, k1=k1, k2=k2,
                b1=np.asarray(b1, np.float32), b2=np.asarray(b2, np.float32),
                iota=iota, iotac=iotac, cfg1=cfg1, cfg2=cfg2, xpad=xpad)
    return meta


def launch_inputs(meta, layer, dense_rows):
    """dense_rows: [npad, DIN] fp16 (x for L1, h for L2)."""
    plan = meta['plan']
    n_cores = meta['n_cores']
    HC = meta['HC1'] if layer == 1 else meta['C2']
    wcat = meta['wcat1'] if layer == 1 else meta['wcat2']
    b = meta['b1'] if layer == 1 else meta['b2']
    in_maps = []
    for c in range(n_cores):
        perm = np.argsort(plan['perm_pos'][c], kind='stable')  # pos -> node
        xp = dense_rows[perm]
        in_maps.append(dict(
            xT=np.ascontiguousarray(xp.T),
            wcat=wcat,
            brep=np.tile(b, (128, 1)).astype(np.float32),
            iota=meta['iota'],
            iotac=meta['iotac'],
            blob=plan['per_core'][c][f'blob{layer}'],
            dt=plan['per_core'][c]['dt'],
        ))
    return in_maps


class SpmdRunner:
    def __init__(self, nc, n_cores=8):
        install_neuronx_cc_hook()
        self.nc = nc
        self.n_cores = n_cores
        partition_name = nc.partition_id_tensor.name if nc.partition_id_tensor else None
        in_names, out_names, out_avals, zero_outs = [], [], [], []
        for alloc in nc.m.functions[0].allocations:
            if not isinstance(alloc, mybir.MemoryLocationSet):
                continue
            name = alloc.memorylocations[0].name
            if alloc.kind == "ExternalInput":
                if name != partition_name:
                    in_names.append(name)
            elif alloc.kind == "ExternalOutput":
                out_names.append(name)
                shape = tuple(alloc.tensor_shape)
                dtype = mybir.dt.np(alloc.dtype)
                out_avals.append(jax.core.ShapedArray(shape, dtype))
                zero_outs.append(np.zeros(shape, dtype))
        self.in_names = list(in_names)
        self.out_names = out_names
        self.out_avals = out_avals
        self.zero_outs = zero_outs
        n_params = len(in_names)
        n_outs = len(out_avals)
        all_in_names = in_names + out_names
        if partition_name is not None:
            all_in_names.append(partition_name)

        def _body(*args):
            operands = list(args)
            if partition_name is not None:
                operands.append(partition_id_tensor())
            outs = _bass_exec_p.bind(
                *operands,
                out_avals=tuple(out_avals),
                in_names=tuple(all_in_names),
                out_names=tuple(out_names),
                lowering_input_output_aliases=(),
                sim_require_finite=False,
                sim_require_nnan=False,
                nc=nc,
            )
            return tuple(outs)

        devices = jax.devices()[:n_cores]
        self.mesh = Mesh(np.asarray(devices), ("core",))
        in_specs = (PartitionSpec("core"),) * (n_params + n_outs)
        out_specs = (PartitionSpec("core"),) * n_outs
        self.fn = jax.jit(
            shard_map(_body, mesh=self.mesh, in_specs=in_specs,
                      out_specs=out_specs, check_rep=False),
            keep_unused=True,
        )
        self._dev_args = None

    def stage(self, in_maps):
        n = self.n_cores
        concat_in = [
            np.concatenate([np.asarray(in_maps[c][name]) for c in range(n)], axis=0)
            for name in self.in_names
        ]
        concat_zeros = [
            np.zeros((n * z.shape[0], *z.shape[1:]), z.dtype) for z in self.zero_outs
        ]
        self._dev_args = [jax.device_put(a) for a in concat_in + concat_zeros]

    def run(self):
        outs = self.fn(*self._dev_args)
        jax.block_until_ready(outs)
        return outs

    def results(self, outs):
        n = self.n_cores
        return [
            {name: np.asarray(outs[i]).reshape(n, *self.out_avals[i].shape)[c]
             for i, name in enumerate(self.out_names)}
            for c in range(n)
        ]

    def time_it(self, iters=5):
        self.run()
        ts = []
        for _ in range(iters):
            t0 = time.perf_counter()
            self.run()
            ts.append(time.perf_counter() - t0)
        return min(ts), ts


def build_floor(cfg, n_units):
    """Trivial kernel with identical I/O decls, for dispatch-floor timing."""
    heads = cfg['heads']
    HC = heads * cfg['ch']
    npad = cfg['npad']
    bpu = cfg['bpu']
    ulen = bpu * 128
    wq = ulen // 16
    BB = 2 * wq + bpu + bpu * heads
    OUTDT = F16 if cfg['out_f16'] else F32
    nc = bacc.Bacc(target_bir_lowering=False)
    xT = nc.dram_tensor("xT", [128, npad], F16, kind="ExternalInput")
    nc.dram_tensor("wcat", [128, HC + 2 * heads], F16, kind="ExternalInput")
    nc.dram_tensor("brep", [128, HC], F32, kind="ExternalInput")
    nc.dram_tensor("iota", [128, 128], F16, kind="ExternalInput")
    nc.dram_tensor("blob", [n_units, 128, BB], I16, kind="ExternalInput")
    outd = nc.dram_tensor("out", [cfg['shard'], cfg['out_cols']], OUTDT,
                          kind="ExternalOutput")
    with tile.TileContext(nc) as tc:
        with tc.tile_pool(name="s", bufs=2) as pool:
            t0 = pool.tile([128, 128], F16)
            nc.sync.dma_start(out=t0[:], in_=xT[:, 0:128])
            t = pool.tile([128, cfg['out_cols']], OUTDT)
            nc.vector.memset(t[:], 0)
            for ci in range(cfg['shard'] // 128):
                nc.sync.dma_start(out=outd[ci * 128:(ci + 1) * 128, :], in_=t[:])
    nc.compile()
    return nc


def kernel(**inputs):
    inputs = {k: np.asarray(v) for k, v in inputs.items()}
    x = inputs['x'].astype(np.float32)
    edge_index = inputs['edge_index'].astype(np.int64)
    ew = inputs['edge_weight'].astype(np.float32)
    meta = gat_prepare(
        x, edge_index, ew,
        inputs['W1'].astype(np.float32), inputs['att_src1'].astype(np.float32),
        inputs['att_dst1'].astype(np.float32), inputs['att_edge1'].astype(np.float32),
        inputs['We1'].astype(np.float32), inputs['b1'].astype(np.float32),
        inputs['W2'].astype(np.float32), inputs['att_src2'].astype(np.float32),
        inputs['att_dst2'].astype(np.float32), inputs['att_edge2'].astype(np.float32),
        inputs['We2'].astype(np.float32), inputs['b2'].astype(np.float32))
    nc1 = build_launch(meta['cfg1'])
    nc2 = build_launch(meta['cfg2'])
    N, shard, n_cores = meta['N'], meta['shard'], meta['n_cores']

    r1 = SpmdRunner(nc1, n_cores)
    r1.stage(launch_inputs(meta, 1, meta['xpad']))
    res1 = r1.results(r1.run())
    hfull = np.concatenate([res1[c]['out'] for c in range(n_cores)], axis=0)
    hfull[N:] = 0

    r2 = SpmdRunner(nc2, n_cores)
    r2.stage(launch_inputs(meta, 2, hfull))
    res2 = r2.results(r2.run())
    out = np.concatenate([res2[c]['out'] for c in range(n_cores)], axis=0)[:N]

    floor_r = None
    try:
        ncf = build_floor(meta['cfg1'], len(meta['plan']['units']))
        floor_r = SpmdRunner(ncf, n_cores)
        floor_r.stage(launch_inputs(meta, 1, meta['xpad']))
        floor_r.run()
    except Exception:
        floor_r = None
    kernel._last = dict(meta=meta, r1=r1, r2=r2, nc1=nc1, nc2=nc2, floor=floor_r)
    return out.astype(np.float32)
